# revision 2
# baseline (speedup 1.0000x reference)
"""Trainium2 Bass CRF loss — overlapping segments, fused super-chain variant.

Same algorithm as kernel_v2 (overlapping-segment forward recurrence in bf16
with a constant leak, boundary-sum markers, indirect-DMA numerator), but the
per-core chains are packed into TWO lockstep "super-chains" to cut
per-instruction overheads (PE wait-queue parking, DVE dispatch, Tile sem
throttling):

  super D: 3 segments side by side, state [128, 384], R3=41 rounds
  super P: 2 segments,              state [128, 256], R2=32 rounds

Each round is ONE matmul + ONE DVE multiply per super-chain (GPSIMD cannot
read PSUM — walrus rejects it — so all multiplies live on DVE; gpsimd only
runs the numerator gathers, fully overlapped under the chains).
"""

import os
import sys

for _p in ("/opt/trn_rl_repo", "/root/.axon_site/_ro/trn_rl_repo"):
    if os.path.isdir(_p) and _p not in sys.path:
        sys.path.append(_p)

from contextlib import ExitStack

import numpy as np

import concourse.bass as bass
import concourse.tile as tile
from concourse import bacc, mybir
from concourse import bass_utils

B, S, T = 128, 1024, 128
LEAK = 5.85
R3, R2 = 41, 32                 # rounds of the 3-seg / 2-seg super-chains
ROUNDS = [R3, R3, R3, R2, R2]   # per-segment-slot rounds (for the plan)
NCHAIN = 5
AR = 11
NCOL = R3 * 384 + R2 * 256
BASE_P = R3 * 384
NB = 16
TSE = T * T + 2 * T + 1
NUM_MS = [0.012 + 0.0025 * i for i in range(16)]  # numerator matmul waves
REDUCE_MS = 0.053
MARK1_MS = 0.024
MARK2_MS = 0.06
MKW = 2048    # marker strip width: bank-aligned sections
# [0:384) A_D | [512:1024) A_P,A12_P | [1024:1280) B_P | [1536:1920) B_D

F32 = mybir.dt.float32
I32 = mybir.dt.int32
BF16 = mybir.dt.bfloat16
AF = mybir.ActivationFunctionType
OP = mybir.AluOpType
NP_BF16 = mybir.dt.np(BF16)

HEAD_TRANS = 0
HEAD_BIAS = 128       # [init_bias, leak]
HEAD_BCOL = 130       # [ones, B_P0, B_P1] raw (pre-exp)
HEAD_INIT = 133
HEAD_W = 133 + 128


def chain_plan():
    chains = []
    for c in range(8):
        for s in range(NCHAIN):
            ch = {"core": c, "slot": s, "rounds": ROUNDS[s]}
            if c == 0 and s == 0:
                ch.update(kind="first", a_round=None, start=1, net=ROUNDS[0])
            else:
                ar = 12 if (s == NCHAIN - 1 and c >= 3) else AR
                ch.update(kind="mid", a_round=ar, net=ROUNDS[s] - 1 - ar)
            chains.append(ch)
    pos = chains[0]["start"] + chains[0]["rounds"] - 1
    for ch in chains[1:]:
        ch["start"] = pos - ch["a_round"]
        pos = ch["start"] + ch["rounds"] - 1
    assert pos == S - 1, pos
    assert sum(ch["net"] for ch in chains) == S - 1
    return chains


CHAINS = chain_plan()


def col_off(slot, r):
    """xt column offset of the 128-col block for (segment slot, round)."""
    if slot < 3:
        return r * 384 + slot * 128
    return BASE_P + r * 256 + (slot - 3) * 128


def chunk_list(total, first, size):
    out = [(0, min(first, total))]
    while out[-1][0] + out[-1][1] < total:
        off = out[-1][0] + out[-1][1]
        out.append((off, min(size, total - off)))
    return out


def build_program(numerator=True, chains=True):
    nc = bacc.Bacc(
        "TRN2",
        target_bir_lowering=False,
        debug=False,
        enable_asserts=False,
        num_devices=8,
    )

    head_d = nc.dram_tensor("head", (T, HEAD_W), F32, kind="ExternalInput")
    xt_d = nc.dram_tensor("xt", (T, NCOL), BF16, kind="ExternalInput")
    ones_d = nc.dram_tensor("ones_bf", (T, 256), BF16, kind="ExternalInput")
    xnb_d = nc.dram_tensor("xnb", (T, NB * 8 * T), BF16, kind="ExternalInput")
    oh_d = nc.dram_tensor("oh", (T, NB * 8 * T), BF16, kind="ExternalInput")
    oh2_d = nc.dram_tensor("oh2", (T, NB * 8 * T), BF16, kind="ExternalInput")
    ident_d = nc.dram_tensor("ident", (T, T), F32, kind="ExternalInput")
    setable_d = nc.dram_tensor("setable", (2 * T, 1), F32, kind="ExternalInput")
    setags_d = nc.dram_tensor("setags", (2 * NB, 1), I32, kind="ExternalInput")

    markers_d = nc.dram_tensor("markers", (1, MKW), F32, kind="ExternalOutput")
    numred_d = nc.dram_tensor("numred", (128, 2), F32, kind="ExternalOutput")
    outse_d = nc.dram_tensor("out_se", (2 * NB, 1), F32, kind="ExternalOutput")

    with ExitStack() as ctx:
        tc = ctx.enter_context(tile.TileContext(nc))
        singles = ctx.enter_context(tc.tile_pool(name="singles", bufs=1))
        raw = ctx.enter_context(tc.tile_pool(name="raw", bufs=3))
        sD = ctx.enter_context(tc.tile_pool(name="sD", bufs=2))
        sP = ctx.enter_context(tc.tile_pool(name="sP", bufs=2))
        pD = ctx.enter_context(tc.tile_pool(name="pD", bufs=1, space="PSUM"))
        pP = ctx.enter_context(tc.tile_pool(name="pP", bufs=1, space="PSUM"))
        mkpool = ctx.enter_context(tc.tile_pool(name="mk", bufs=1, space="PSUM"))
        pnum = ctx.enter_context(tc.tile_pool(name="pnum", bufs=1, space="PSUM"))

        head_sb = singles.tile([T, HEAD_W], F32)
        nc.sync.dma_start(out=head_sb, in_=head_d.ap())
        trans_sb = head_sb[:, HEAD_TRANS:HEAD_TRANS + 128]
        biascol = head_sb[:, HEAD_BIAS:HEAD_BIAS + 2]
        bcolraw = head_sb[:, HEAD_BCOL:HEAD_BCOL + 3]
        initraw = head_sb[:, HEAD_INIT:HEAD_INIT + 128]

        e_bf = singles.tile([T, T], BF16)
        nc.scalar.activation(e_bf, trans_sb, AF.Exp)
        bcol_bf = singles.tile([T, 3], BF16)
        nc.scalar.activation(bcol_bf, bcolraw, AF.Exp)

        # ---- inits ---------------------------------------------------------
        stD = sD.tile([T, 384], BF16, name="aD")
        stP = sP.tile([T, 256], BF16, name="aP")
        nc.scalar.activation(stD[:, 0:128], initraw, AF.Exp, bias=biascol[:, 0:1])
        nc.sync.dma_start(out=stD[:, 128:384], in_=ones_d.ap())
        nc.sync.dma_start(out=stP, in_=ones_d.ap())

        # ---- xhat staging --------------------------------------------------
        xhat = singles.tile([T, NCOL], BF16)
        plan = []
        for off, sz in chunk_list(BASE_P, 1152, 2304):
            plan.append(("D", off, sz, off / 384.0))
        for off, sz in chunk_list(NCOL - BASE_P, 768, 2048):
            plan.append(("P", BASE_P + off, sz, off / 256.0))
        plan.sort(key=lambda t: t[3])

        for _, off, sz, rr in plan:
            need = max(0.0, 3000.0 + rr * 1000.0 - 3500.0)
            with tc.tile_wait_until(need * 1e-6):
                rawc = raw.tile([T, sz], BF16, name="rawc")
                nc.sync.dma_start(out=rawc, in_=xt_d.ap()[:, off:off + sz])
                nc.scalar.activation(
                    xhat[:, off:off + sz], rawc, AF.Exp, bias=biascol[:, 1:2],
                )

        if numerator:
            # one-hot + emission tiles: [128, NB*8*128]; column block (b,k)
            # holds rows s = k*128..k*128+127 of batch row b (s%128 on the
            # partition axis)
            oh_sb = singles.tile([T, NB * 8 * T], BF16)
            oh2_sb = singles.tile([T, NB * 8 * T], BF16)
            xnb_sb = singles.tile([T, NB * 8 * T], BF16)
            ident_sb = singles.tile([T, T], F32)
            nc.sync.dma_start(out=ident_sb, in_=ident_d.ap())
            setags_sb = singles.tile([2 * NB, 1], I32)
            nc.sync.dma_start(out=setags_sb, in_=setags_d.ap())
            cp_ps = pnum.tile([T, T], F32)
            em_ps = pnum.tile([T, T], F32)
            se_sb = singles.tile([2 * NB, 1], F32)
            nwave = len(NUM_MS)
            per = NB * 8 * T // nwave
            for i in range(nwave):
                with tc.tile_wait_until(NUM_MS[i]):
                    sl = slice(i * per, (i + 1) * per)
                    nc.sync.dma_start(out=oh_sb[:, sl], in_=oh_d.ap()[:, sl])
                    nc.sync.dma_start(out=oh2_sb[:, sl], in_=oh2_d.ap()[:, sl])
                    nc.sync.dma_start(out=xnb_sb[:, sl], in_=xnb_d.ap()[:, sl])
            with tc.tile_wait_until(REDUCE_MS - 0.006):
                n_em = n_cp = 0
                for b in range(NB):
                    for k in range(8):
                        o = (b * 8 + k) * T
                        otile = oh_sb[:, o:o + T]
                        nc.tensor.matmul(
                            em_ps, lhsT=otile, rhs=xnb_sb[:, o:o + T],
                            start=(n_em == 0), stop=(n_em == NB * 8 - 1),
                            skip_group_check=True,
                        )
                        n_em += 1
                        nc.tensor.matmul(
                            cp_ps, lhsT=oh_sb[:, o:o + T],
                            rhs=oh2_sb[:, o:o + T],
                            start=(n_cp == 0), stop=(b == NB - 1 and k == 7),
                            skip_group_check=True,
                        )
                        n_cp += 1
                # close the cp accumulation group (last element was emitted
                # above with stop=last for b=NB-1,k=6; the k=7 main matmul of
                # b=NB-1 came before it, so the group is closed correctly)
                nc.gpsimd.indirect_dma_start(
                    out=se_sb, out_offset=None, in_=setable_d.ap(),
                    in_offset=bass.IndirectOffsetOnAxis(ap=setags_sb[:, 0:1], axis=0),
                )
                nc.sync.dma_start(out=outse_d.ap(), in_=se_sb)

        # ---- the two super-chains -----------------------------------------
        markers_sb = singles.tile([1, MKW], F32)
        nc.vector.memset(markers_sb, 0.0)
        mkps = mkpool.tile([1, MKW], F32)
        if chains:
            for r in range(R3):
                if r < R2:
                    ps = pP.tile([T, 256], F32, name="pP")
                    nc.tensor.matmul(ps, lhsT=e_bf, rhs=stP, start=True, stop=True)
                    nst = sP.tile([T, 256], BF16, name="aP")
                    nc.vector.tensor_tensor(
                        nst, ps, xhat[:, BASE_P + r * 256:BASE_P + (r + 1) * 256],
                        op=OP.mult,
                    )
                    stP = nst
                    if r == AR:
                        nc.tensor.matmul(
                            mkps[:, 512:768], lhsT=bcol_bf[:, 0:1],
                            rhs=stP, start=True, stop=True,
                        )
                    elif r == 12:
                        nc.tensor.matmul(
                            mkps[:, 768:1024], lhsT=bcol_bf[:, 0:1],
                            rhs=stP, start=True, stop=True,
                        )
                    elif r == R2 - 1:
                        nc.tensor.matmul(
                            mkps[:, 1024:1152], lhsT=bcol_bf[:, 1:2],
                            rhs=stP[:, 0:128], start=True, stop=True,
                        )
                        nc.tensor.matmul(
                            mkps[:, 1152:1280], lhsT=bcol_bf[:, 2:3],
                            rhs=stP[:, 128:256], start=True, stop=True,
                        )
                psd = pD.tile([T, 384], F32, name="pD")
                nc.tensor.matmul(psd, lhsT=e_bf, rhs=stD, start=True, stop=True)
                nstd = sD.tile([T, 384], BF16, name="aD")
                nc.vector.tensor_tensor(
                    nstd, psd, xhat[:, r * 384:(r + 1) * 384], op=OP.mult,
                )
                stD = nstd
                if r == AR:
                    nc.tensor.matmul(
                        mkps[:, 0:384], lhsT=bcol_bf[:, 0:1],
                        rhs=stD, start=True, stop=True,
                    )
                elif r == R3 - 1:
                    nc.tensor.matmul(
                        mkps[:, 1536:1920], lhsT=bcol_bf[:, 0:1],
                        rhs=stD, start=True, stop=True,
                    )
            with tc.tile_wait_until(MARK1_MS):
                nc.scalar.activation(markers_sb[:, 0:384], mkps[:, 0:384], AF.Copy)
                nc.scalar.activation(markers_sb[:, 512:1280], mkps[:, 512:1280], AF.Copy)
            with tc.tile_wait_until(MARK2_MS):
                nc.scalar.activation(markers_sb[:, 1536:1920], mkps[:, 1536:1920], AF.Copy)
                nc.sync.dma_start(out=markers_d.ap(), in_=markers_sb)

        if numerator:
            numred_sb = singles.tile([128, 2], F32)
            scr = singles.tile([128, T], F32)
            scr2 = singles.tile([128, T], F32)
            with tc.tile_wait_until(REDUCE_MS):
                nc.vector.tensor_tensor(scr, em_ps, ident_sb, op=OP.mult)
                nc.vector.reduce_sum(out=numred_sb[:, 0:1], in_=scr, axis=mybir.AxisListType.X)
                nc.vector.tensor_tensor(scr2, cp_ps, trans_sb, op=OP.mult)
                nc.vector.reduce_sum(out=numred_sb[:, 1:2], in_=scr2, axis=mybir.AxisListType.X)
                nc.sync.dma_start(out=numred_d.ap(), in_=numred_sb)

    nc.compile()
    return nc


def prepare_in_maps(inputs):
    x = np.asarray(inputs["inputs"], dtype=np.float32)
    tags = np.asarray(inputs["tags"]).astype(np.int64)
    trans = np.ascontiguousarray(np.asarray(inputs["transitions"], np.float32))
    start = np.asarray(inputs["start_transitions"], np.float32)
    end = np.asarray(inputs["end_transitions"], np.float32)
    x_bf = x.astype(NP_BF16)

    in_maps = []
    for c in range(8):
        head = np.zeros((T, HEAD_W), np.float32)
        head[:, HEAD_TRANS:HEAD_TRANS + 128] = trans
        head[:, HEAD_BIAS + 1] = -LEAK
        xt = np.zeros((T, NCOL), NP_BF16)
        for ch in CHAINS:
            if ch["core"] != c:
                continue
            s = ch["slot"]
            ps, R = ch["start"], ch["rounds"]
            blk = x_bf[:, ps:ps + R, :].transpose(2, 1, 0)   # [T, R, B]
            for r in range(R):
                o = col_off(s, r)
                xt[:, o:o + 128] = blk[:, r, :]
            if ch["kind"] == "first":
                head[:, HEAD_INIT:HEAD_INIT + 128] = x[:, 0, :].T
                head[:, HEAD_BIAS] = start
            if ch is CHAINS[-1]:
                head[:, HEAD_BCOL + 2] = end
        in_map = {"head": head, "xt": xt,
                  "ones_bf": np.ones((T, 256), NP_BF16)}

        tb = tags[c * NB:(c + 1) * NB]              # [16, 1024]
        oh = (tb[:, :, None] == np.arange(T)[None, None, :])
        tbn = np.concatenate([tb[:, 1:], np.full((NB, 1), -1, np.int64)], axis=1)
        oh2 = (tbn[:, :, None] == np.arange(T)[None, None, :])
        oh = oh.reshape(NB, 8, 128, T).transpose(2, 0, 1, 3).reshape(128, NB * 8 * T)
        oh2 = oh2.reshape(NB, 8, 128, T).transpose(2, 0, 1, 3).reshape(128, NB * 8 * T)
        xn = x_bf[c * NB:(c + 1) * NB].reshape(NB, 8, 128, T)
        xn = xn.transpose(2, 0, 1, 3).reshape(128, NB * 8 * T)
        setags = np.concatenate(
            [tb[:, 0], T + tb[:, S - 1]]
        ).reshape(2 * NB, 1).astype(np.int32)
        in_map.update({
            "oh": np.ascontiguousarray(oh.astype(NP_BF16)),
            "oh2": np.ascontiguousarray(oh2.astype(NP_BF16)),
            "xnb": np.ascontiguousarray(xn),
            "ident": np.eye(T, dtype=np.float32),
            "setable": np.concatenate([start, end]).reshape(2 * T, 1).astype(np.float32),
            "setags": setags,
        })
        in_maps.append(in_map)
    return in_maps


def assemble(results):
    den_tot = np.float64(0.0)
    for ch in CHAINS:
        mkf = results[ch["core"]]["markers"].reshape(MKW).astype(np.float64)
        s = ch["slot"]
        if s < 3:
            nA = mkf[128 * s:128 * (s + 1)]
            nB = mkf[1536 + 128 * s:1536 + 128 * (s + 1)]
        else:
            k = s - 3
            a0 = 512 + 128 * k
            a1 = 768 + 128 * k
            nA = mkf[a0:a0 + 128] if ch["a_round"] == AR else mkf[a1:a1 + 128]
            nB = mkf[1024 + 128 * k:1024 + 128 * (k + 1)]
        if ch["kind"] == "first":
            den_tot += (np.log(nB) + LEAK * ch["rounds"]).sum()
        else:
            nsteps = ch["rounds"] - 1 - ch["a_round"]
            den_tot += (np.log(nB) - np.log(nA) + LEAK * nsteps).sum()

    num_tot = np.float64(0.0)
    for c in range(8):
        num_tot += results[c]["numred"].astype(np.float64).sum()
        num_tot += results[c]["out_se"].astype(np.float64).sum()
    return np.asarray(num_tot - den_tot, dtype=np.float32)


_CACHE = {}


def kernel(**inputs):
    if "nc" not in _CACHE:
        _CACHE["nc"] = build_program()
    nc = _CACHE["nc"]
    in_maps = prepare_in_maps(inputs)
    res = bass_utils.run_bass_kernel_spmd(nc, in_maps, core_ids=list(range(8)))
    return assemble(res.results)


# revision 3
# speedup vs baseline: 1.2148x; 1.2148x over previous
"""Trainium2 Bass CRF loss — overlapping segments, fused super-chain variant.

Same algorithm as kernel_v2 (overlapping-segment forward recurrence in bf16
with a constant leak, boundary-sum markers, indirect-DMA numerator), but the
per-core chains are packed into TWO lockstep "super-chains" to cut
per-instruction overheads (PE wait-queue parking, DVE dispatch, Tile sem
throttling):

  super D: 3 segments side by side, state [128, 384], R3=41 rounds
  super P: 2 segments,              state [128, 256], R2=32 rounds

Each round is ONE matmul + ONE DVE multiply per super-chain (GPSIMD cannot
read PSUM — walrus rejects it — so all multiplies live on DVE; gpsimd only
runs the numerator gathers, fully overlapped under the chains).
"""

import os
import sys

for _p in ("/opt/trn_rl_repo", "/root/.axon_site/_ro/trn_rl_repo"):
    if os.path.isdir(_p) and _p not in sys.path:
        sys.path.append(_p)

from contextlib import ExitStack

import numpy as np

import concourse.bass as bass
import concourse.tile as tile
from concourse import bacc, mybir
from concourse import bass_utils

B, S, T = 128, 1024, 128
LEAK = 5.85
R3, R2 = 35, 41                 # rounds of the 3-seg / 2-seg super-chains
ROUNDS = [R3, R3, R3, R2, R2]   # per-segment-slot rounds (for the plan)
NCHAIN = 5
AR = 11
NCOL = R3 * 384 + R2 * 256
BASE_P = R3 * 384
NB = 16
TSE = T * T + 2 * T + 1
NUM_MS = [0.004 + 0.001 * i for i in range(16)]  # numerator DMA waves
NUMMM_MS = [0.022 + 0.0016 * i for i in range(16)]  # numerator matmul waves
REDUCE_MS = 0.046
MARK1_MS = 0.024
MARK2_MS = 0.0425
MKW = 2048    # marker strip width: bank-aligned sections
# [0:384) A_D | [512:1024) A_P,A12_P | [1024:1280) B_P | [1536:1920) B_D

F32 = mybir.dt.float32
I32 = mybir.dt.int32
BF16 = mybir.dt.bfloat16
AF = mybir.ActivationFunctionType
OP = mybir.AluOpType
NP_BF16 = mybir.dt.np(BF16)
FP8 = mybir.dt.float8e4
NP_FP8 = mybir.dt.np(FP8)

HEAD_TRANS = 0
HEAD_BIAS = 128       # [init_bias, leak]
HEAD_BCOL = 130       # [ones, B_P0, B_P1] raw (pre-exp)
HEAD_INIT = 133
HEAD_W = 133 + 128


def chain_plan():
    chains = []
    for c in range(8):
        for s in range(NCHAIN):
            ch = {"core": c, "slot": s, "rounds": ROUNDS[s]}
            if c == 0 and s == 0:
                ch.update(kind="first", a_round=None, start=1, net=ROUNDS[0])
            else:
                ar = 12 if (s == NCHAIN - 1 and c >= 3) else AR
                ch.update(kind="mid", a_round=ar, net=ROUNDS[s] - 1 - ar)
            chains.append(ch)
    pos = chains[0]["start"] + chains[0]["rounds"] - 1
    for ch in chains[1:]:
        ch["start"] = pos - ch["a_round"]
        pos = ch["start"] + ch["rounds"] - 1
    assert pos == S - 1, pos
    assert sum(ch["net"] for ch in chains) == S - 1
    return chains


CHAINS = chain_plan()


def col_off(slot, r):
    """xt column offset of the 128-col block for (segment slot, round)."""
    if slot < 3:
        return r * 384 + slot * 128
    return BASE_P + r * 256 + (slot - 3) * 128


def chunk_list(total, first, size):
    out = [(0, min(first, total))]
    while out[-1][0] + out[-1][1] < total:
        off = out[-1][0] + out[-1][1]
        out.append((off, min(size, total - off)))
    return out


def build_program(numerator=True, chains=True):
    nc = bacc.Bacc(
        "TRN2",
        target_bir_lowering=False,
        debug=False,
        enable_asserts=False,
        num_devices=8,
    )

    head_d = nc.dram_tensor("head", (T, HEAD_W), F32, kind="ExternalInput")
    xt_d = nc.dram_tensor("xt", (T, NCOL), BF16, kind="ExternalInput")
    ones_d = nc.dram_tensor("ones_bf", (T, 256), BF16, kind="ExternalInput")
    xnb_d = nc.dram_tensor("xnb", (T, NB * 8 * T), FP8, kind="ExternalInput")
    oh_d = nc.dram_tensor("oh", (T, NB * 8 * T), FP8, kind="ExternalInput")
    oh2_d = nc.dram_tensor("oh2", (T, NB * 8 * T), FP8, kind="ExternalInput")
    ident_d = nc.dram_tensor("ident", (T, T), F32, kind="ExternalInput")
    setable_d = nc.dram_tensor("setable", (2 * T, 1), F32, kind="ExternalInput")
    setags_d = nc.dram_tensor("setags", (2 * NB, 1), I32, kind="ExternalInput")

    markers_d = nc.dram_tensor("markers", (1, MKW), F32, kind="ExternalOutput")
    numred_d = nc.dram_tensor("numred", (128, 2), F32, kind="ExternalOutput")
    outse_d = nc.dram_tensor("out_se", (2 * NB, 1), F32, kind="ExternalOutput")

    with ExitStack() as ctx:
        tc = ctx.enter_context(tile.TileContext(nc))
        singles = ctx.enter_context(tc.tile_pool(name="singles", bufs=1))
        raw = ctx.enter_context(tc.tile_pool(name="raw", bufs=3))
        sD = ctx.enter_context(tc.tile_pool(name="sD", bufs=2))
        sP = ctx.enter_context(tc.tile_pool(name="sP", bufs=2))
        pD = ctx.enter_context(tc.tile_pool(name="pD", bufs=1, space="PSUM"))
        pP = ctx.enter_context(tc.tile_pool(name="pP", bufs=1, space="PSUM"))
        mkpool = ctx.enter_context(tc.tile_pool(name="mk", bufs=1, space="PSUM"))
        pnum = ctx.enter_context(tc.tile_pool(name="pnum", bufs=1, space="PSUM"))

        head_sb = singles.tile([T, HEAD_W], F32)
        nc.sync.dma_start(out=head_sb, in_=head_d.ap())
        trans_sb = head_sb[:, HEAD_TRANS:HEAD_TRANS + 128]
        biascol = head_sb[:, HEAD_BIAS:HEAD_BIAS + 2]
        bcolraw = head_sb[:, HEAD_BCOL:HEAD_BCOL + 3]
        initraw = head_sb[:, HEAD_INIT:HEAD_INIT + 128]

        e_bf = singles.tile([T, T], BF16)
        nc.scalar.activation(e_bf, trans_sb, AF.Exp)
        bcol_bf = singles.tile([T, 3], BF16)
        nc.scalar.activation(bcol_bf, bcolraw, AF.Exp)

        # ---- inits ---------------------------------------------------------
        stD = sD.tile([T, 384], BF16, name="aD")
        stP = sP.tile([T, 256], BF16, name="aP")
        nc.scalar.activation(stD[:, 0:128], initraw, AF.Exp, bias=biascol[:, 0:1])
        nc.sync.dma_start(out=stD[:, 128:384], in_=ones_d.ap())
        nc.sync.dma_start(out=stP, in_=ones_d.ap())

        # ---- xhat staging --------------------------------------------------
        xhat = singles.tile([T, NCOL], BF16)
        plan = []
        for off, sz in chunk_list(BASE_P, 768, 2304):
            plan.append(("D", off, sz, off / 384.0))
        for off, sz in chunk_list(NCOL - BASE_P, 512, 2048):
            plan.append(("P", BASE_P + off, sz, off / 256.0))
        plan.sort(key=lambda t: t[3])

        for _, off, sz, rr in plan:
            need = max(0.0, 3000.0 + rr * 1000.0 - 3500.0)
            with tc.tile_wait_until(need * 1e-6):
                rawc = raw.tile([T, sz], BF16, name="rawc")
                nc.sync.dma_start(out=rawc, in_=xt_d.ap()[:, off:off + sz])
                nc.scalar.activation(
                    xhat[:, off:off + sz], rawc, AF.Exp, bias=biascol[:, 1:2],
                )

        if numerator:
            # one-hot + emission tiles: [128, NB*8*128]; column block (b,k)
            # holds rows s = k*128..k*128+127 of batch row b (s%128 on the
            # partition axis)
            oh_sb = singles.tile([T, NB * 8 * T], FP8)
            oh2_sb = singles.tile([T, NB * 8 * T], FP8)
            xnb_sb = singles.tile([T, NB * 8 * T], FP8)
            ident_sb = singles.tile([T, T], F32)
            nc.sync.dma_start(out=ident_sb, in_=ident_d.ap())
            setags_sb = singles.tile([2 * NB, 1], I32)
            nc.sync.dma_start(out=setags_sb, in_=setags_d.ap())
            cp_ps = pnum.tile([T, T], F32)
            em_ps = pnum.tile([T, T], F32)
            se_sb = singles.tile([2 * NB, 1], F32)
            nwave = len(NUM_MS)
            per = NB * 8 * T // nwave
            for i in range(nwave):
                with tc.tile_wait_until(NUM_MS[i]):
                    sl = slice(i * per, (i + 1) * per)
                    nc.sync.dma_start(out=oh_sb[:, sl], in_=oh_d.ap()[:, sl])
                    nc.sync.dma_start(out=oh2_sb[:, sl], in_=oh2_d.ap()[:, sl])
                    nc.sync.dma_start(out=xnb_sb[:, sl], in_=xnb_d.ap()[:, sl])
            n_em = n_cp = 0
            for b in range(NB):
                with tc.tile_wait_until(NUMMM_MS[b]):
                    for k in range(8):
                        o = (b * 8 + k) * T
                        otile = oh_sb[:, o:o + T]
                        nc.tensor.matmul(
                            em_ps, lhsT=otile, rhs=xnb_sb[:, o:o + T],
                            start=(n_em == 0), stop=(n_em == NB * 8 - 1),
                            skip_group_check=True,
                        )
                        n_em += 1
                        nc.tensor.matmul(
                            cp_ps, lhsT=oh_sb[:, o:o + T],
                            rhs=oh2_sb[:, o:o + T],
                            start=(n_cp == 0), stop=(b == NB - 1 and k == 7),
                            skip_group_check=True,
                        )
                        n_cp += 1
            with tc.tile_wait_until(REDUCE_MS - 0.004):
                nc.gpsimd.indirect_dma_start(
                    out=se_sb, out_offset=None, in_=setable_d.ap(),
                    in_offset=bass.IndirectOffsetOnAxis(ap=setags_sb[:, 0:1], axis=0),
                )
                nc.sync.dma_start(out=outse_d.ap(), in_=se_sb)

        # ---- the two super-chains -----------------------------------------
        markers_sb = singles.tile([1, MKW], F32)
        nc.vector.memset(markers_sb, 0.0)
        mkps = mkpool.tile([1, MKW], F32)
        if chains:
            for r in range(max(R3, R2)):
                if r < R2:
                    ps = pP.tile([T, 256], F32, name="pP")
                    nc.tensor.matmul(ps, lhsT=e_bf, rhs=stP, start=True, stop=True)
                    nst = sP.tile([T, 256], BF16, name="aP")
                    nc.vector.tensor_tensor(
                        nst, ps, xhat[:, BASE_P + r * 256:BASE_P + (r + 1) * 256],
                        op=OP.mult,
                    )
                    stP = nst
                    if r == AR:
                        nc.tensor.matmul(
                            mkps[:, 512:768], lhsT=bcol_bf[:, 0:1],
                            rhs=stP, start=True, stop=True,
                        )
                    elif r == 12:
                        nc.tensor.matmul(
                            mkps[:, 768:1024], lhsT=bcol_bf[:, 0:1],
                            rhs=stP, start=True, stop=True,
                        )
                    elif r == R2 - 1:
                        nc.tensor.matmul(
                            mkps[:, 1024:1152], lhsT=bcol_bf[:, 1:2],
                            rhs=stP[:, 0:128], start=True, stop=True,
                        )
                        nc.tensor.matmul(
                            mkps[:, 1152:1280], lhsT=bcol_bf[:, 2:3],
                            rhs=stP[:, 128:256], start=True, stop=True,
                        )
                if r >= R3:
                    continue
                psd = pD.tile([T, 384], F32, name="pD")
                nc.tensor.matmul(psd, lhsT=e_bf, rhs=stD, start=True, stop=True)
                nstd = sD.tile([T, 384], BF16, name="aD")
                nc.vector.tensor_tensor(
                    nstd, psd, xhat[:, r * 384:(r + 1) * 384], op=OP.mult,
                )
                stD = nstd
                if r == AR:
                    nc.tensor.matmul(
                        mkps[:, 0:384], lhsT=bcol_bf[:, 0:1],
                        rhs=stD, start=True, stop=True,
                    )
                elif r == R3 - 1:
                    nc.tensor.matmul(
                        mkps[:, 1536:1920], lhsT=bcol_bf[:, 0:1],
                        rhs=stD, start=True, stop=True,
                    )
            with tc.tile_wait_until(MARK1_MS):
                nc.scalar.activation(markers_sb[:, 0:384], mkps[:, 0:384], AF.Copy)
                nc.scalar.activation(markers_sb[:, 512:1024], mkps[:, 512:1024], AF.Copy)
            with tc.tile_wait_until(MARK2_MS):
                nc.scalar.activation(markers_sb[:, 1024:1280], mkps[:, 1024:1280], AF.Copy)
                nc.scalar.activation(markers_sb[:, 1536:1920], mkps[:, 1536:1920], AF.Copy)
                nc.sync.dma_start(out=markers_d.ap(), in_=markers_sb)

        if numerator:
            numred_sb = singles.tile([128, 2], F32)
            scr = singles.tile([128, T], F32)
            scr2 = singles.tile([128, T], F32)
            with tc.tile_wait_until(REDUCE_MS):
                nc.vector.tensor_tensor(scr, em_ps, ident_sb, op=OP.mult)
                nc.vector.reduce_sum(out=numred_sb[:, 0:1], in_=scr, axis=mybir.AxisListType.X)
                nc.vector.tensor_tensor(scr2, cp_ps, trans_sb, op=OP.mult)
                nc.vector.reduce_sum(out=numred_sb[:, 1:2], in_=scr2, axis=mybir.AxisListType.X)
                nc.sync.dma_start(out=numred_d.ap(), in_=numred_sb)

    nc.compile()
    return nc


def prepare_in_maps(inputs):
    x = np.asarray(inputs["inputs"], dtype=np.float32)
    tags = np.asarray(inputs["tags"]).astype(np.int64)
    trans = np.ascontiguousarray(np.asarray(inputs["transitions"], np.float32))
    start = np.asarray(inputs["start_transitions"], np.float32)
    end = np.asarray(inputs["end_transitions"], np.float32)
    x_bf = x.astype(NP_BF16)

    in_maps = []
    for c in range(8):
        head = np.zeros((T, HEAD_W), np.float32)
        head[:, HEAD_TRANS:HEAD_TRANS + 128] = trans
        head[:, HEAD_BIAS + 1] = -LEAK
        xt = np.zeros((T, NCOL), NP_BF16)
        for ch in CHAINS:
            if ch["core"] != c:
                continue
            s = ch["slot"]
            ps, R = ch["start"], ch["rounds"]
            blk = x_bf[:, ps:ps + R, :].transpose(2, 1, 0)   # [T, R, B]
            for r in range(R):
                o = col_off(s, r)
                xt[:, o:o + 128] = blk[:, r, :]
            if ch["kind"] == "first":
                head[:, HEAD_INIT:HEAD_INIT + 128] = x[:, 0, :].T
                head[:, HEAD_BIAS] = start
            if ch is CHAINS[-1]:
                head[:, HEAD_BCOL + 2] = end
        in_map = {"head": head, "xt": xt,
                  "ones_bf": np.ones((T, 256), NP_BF16)}

        tb = tags[c * NB:(c + 1) * NB]              # [16, 1024]
        oh = (tb[:, :, None] == np.arange(T)[None, None, :])
        tbn = np.concatenate([tb[:, 1:], np.full((NB, 1), -1, np.int64)], axis=1)
        oh2 = (tbn[:, :, None] == np.arange(T)[None, None, :])
        oh = oh.reshape(NB, 8, 128, T).transpose(2, 0, 1, 3).reshape(128, NB * 8 * T)
        oh2 = oh2.reshape(NB, 8, 128, T).transpose(2, 0, 1, 3).reshape(128, NB * 8 * T)
        xn = x[c * NB:(c + 1) * NB].reshape(NB, 8, 128, T)
        xn = xn.transpose(2, 0, 1, 3).reshape(128, NB * 8 * T)
        setags = np.concatenate(
            [tb[:, 0], T + tb[:, S - 1]]
        ).reshape(2 * NB, 1).astype(np.int32)
        in_map.update({
            "oh": np.ascontiguousarray(oh.astype(NP_FP8)),
            "oh2": np.ascontiguousarray(oh2.astype(NP_FP8)),
            "xnb": np.ascontiguousarray(xn.astype(NP_FP8)),
            "ident": np.eye(T, dtype=np.float32),
            "setable": np.concatenate([start, end]).reshape(2 * T, 1).astype(np.float32),
            "setags": setags,
        })
        in_maps.append(in_map)
    return in_maps


def assemble(results):
    den_tot = np.float64(0.0)
    for ch in CHAINS:
        mkf = results[ch["core"]]["markers"].reshape(MKW).astype(np.float64)
        s = ch["slot"]
        if s < 3:
            nA = mkf[128 * s:128 * (s + 1)]
            nB = mkf[1536 + 128 * s:1536 + 128 * (s + 1)]
        else:
            k = s - 3
            a0 = 512 + 128 * k
            a1 = 768 + 128 * k
            nA = mkf[a0:a0 + 128] if ch["a_round"] == AR else mkf[a1:a1 + 128]
            nB = mkf[1024 + 128 * k:1024 + 128 * (k + 1)]
        if ch["kind"] == "first":
            den_tot += (np.log(nB) + LEAK * ch["rounds"]).sum()
        else:
            nsteps = ch["rounds"] - 1 - ch["a_round"]
            den_tot += (np.log(nB) - np.log(nA) + LEAK * nsteps).sum()

    num_tot = np.float64(0.0)
    for c in range(8):
        num_tot += results[c]["numred"].astype(np.float64).sum()
        num_tot += results[c]["out_se"].astype(np.float64).sum()
    return np.asarray(num_tot - den_tot, dtype=np.float32)


_CACHE = {}


def kernel(**inputs):
    if "nc" not in _CACHE:
        _CACHE["nc"] = build_program()
    nc = _CACHE["nc"]
    in_maps = prepare_in_maps(inputs)
    res = bass_utils.run_bass_kernel_spmd(nc, in_maps, core_ids=list(range(8)))
    return assemble(res.results)


# revision 4
# speedup vs baseline: 1.2768x; 1.0510x over previous
"""Trainium2 Bass CRF loss — overlapping segments, fused super-chain variant.

Same algorithm as kernel_v2 (overlapping-segment forward recurrence in bf16
with a constant leak, boundary-sum markers, indirect-DMA numerator), but the
per-core chains are packed into TWO lockstep "super-chains" to cut
per-instruction overheads (PE wait-queue parking, DVE dispatch, Tile sem
throttling):

  super D: 3 segments side by side, state [128, 384], R3=41 rounds
  super P: 2 segments,              state [128, 256], R2=32 rounds

Each round is ONE matmul + ONE DVE multiply per super-chain (GPSIMD cannot
read PSUM — walrus rejects it — so all multiplies live on DVE; gpsimd only
runs the numerator gathers, fully overlapped under the chains).
"""

import os
import sys

for _p in ("/opt/trn_rl_repo", "/root/.axon_site/_ro/trn_rl_repo"):
    if os.path.isdir(_p) and _p not in sys.path:
        sys.path.append(_p)

from contextlib import ExitStack

import numpy as np

import concourse.bass as bass
import concourse.tile as tile
from concourse import bacc, mybir
from concourse import bass_utils

B, S, T = 128, 1024, 128
LEAK = 5.85
R3, R2 = 33, 34                 # rounds of the 3-seg / 2-seg super-chains
ROUNDS = [R3, R3, R3, R2, R2]   # per-segment-slot rounds (for the plan)
NCHAIN = 5
AR = 7
NCOL = R3 * 384 + R2 * 256
BASE_P = R3 * 384
NB = 16
TSE = T * T + 2 * T + 1
NUM_MS = [0.004 + 0.001 * i for i in range(16)]  # numerator DMA waves
NUMMM_MS = [0.014 + 0.0014 * i for i in range(16)]  # numerator matmul waves
REDUCE_MS = 0.038
MARK1_MS = 0.012
MARK2_MS = 0.036
MKW = 2048    # marker strip width: bank-aligned sections
# [0:384) A_D | [512:1024) A_P,A12_P | [1024:1280) B_P | [1536:1920) B_D

F32 = mybir.dt.float32
I32 = mybir.dt.int32
BF16 = mybir.dt.bfloat16
AF = mybir.ActivationFunctionType
OP = mybir.AluOpType
NP_BF16 = mybir.dt.np(BF16)
FP8 = mybir.dt.float8e4
NP_FP8 = mybir.dt.np(FP8)

HEAD_TRANS = 0
HEAD_BIAS = 128       # [init_bias, leak]
HEAD_BCOL = 130       # [ones, B_P0, B_P1] raw (pre-exp)
HEAD_INIT = 133
HEAD_W = 133 + 128


def chain_plan():
    chains = []
    for c in range(8):
        for s in range(NCHAIN):
            ch = {"core": c, "slot": s, "rounds": ROUNDS[s]}
            if c == 0 and s == 0:
                ch.update(kind="first", a_round=None, start=1, net=ROUNDS[0])
            else:
                ar = 8 if (s == NCHAIN - 1 and c == 7) else AR
                ch.update(kind="mid", a_round=ar, net=ROUNDS[s] - 1 - ar)
            chains.append(ch)
    pos = chains[0]["start"] + chains[0]["rounds"] - 1
    for ch in chains[1:]:
        ch["start"] = pos - ch["a_round"]
        pos = ch["start"] + ch["rounds"] - 1
    assert pos == S - 1, pos
    assert sum(ch["net"] for ch in chains) == S - 1
    return chains


CHAINS = chain_plan()


def col_off(slot, r):
    """xt column offset of the 128-col block for (segment slot, round)."""
    if slot < 3:
        return r * 384 + slot * 128
    return BASE_P + r * 256 + (slot - 3) * 128


def chunk_list(total, first, size):
    out = [(0, min(first, total))]
    while out[-1][0] + out[-1][1] < total:
        off = out[-1][0] + out[-1][1]
        out.append((off, min(size, total - off)))
    return out


def build_program(numerator=True, chains=True):
    nc = bacc.Bacc(
        "TRN2",
        target_bir_lowering=False,
        debug=False,
        enable_asserts=False,
        num_devices=8,
    )

    head_d = nc.dram_tensor("head", (T, HEAD_W), F32, kind="ExternalInput")
    xt_d = nc.dram_tensor("xt", (T, NCOL), BF16, kind="ExternalInput")
    ones_d = nc.dram_tensor("ones_bf", (T, 256), BF16, kind="ExternalInput")
    xnb_d = nc.dram_tensor("xnb", (T, NB * 8 * T), FP8, kind="ExternalInput")
    oh_d = nc.dram_tensor("oh", (T, NB * 8 * T), FP8, kind="ExternalInput")
    oh2_d = nc.dram_tensor("oh2", (T, NB * 8 * T), FP8, kind="ExternalInput")
    ident_d = nc.dram_tensor("ident", (T, T), F32, kind="ExternalInput")
    setable_d = nc.dram_tensor("setable", (2 * T, 1), F32, kind="ExternalInput")
    setags_d = nc.dram_tensor("setags", (2 * NB, 1), I32, kind="ExternalInput")

    markers_d = nc.dram_tensor("markers", (1, MKW), F32, kind="ExternalOutput")
    numred_d = nc.dram_tensor("numred", (128, 2), F32, kind="ExternalOutput")
    outse_d = nc.dram_tensor("out_se", (2 * NB, 1), F32, kind="ExternalOutput")

    with ExitStack() as ctx:
        tc = ctx.enter_context(tile.TileContext(nc))
        singles = ctx.enter_context(tc.tile_pool(name="singles", bufs=1))
        raw = ctx.enter_context(tc.tile_pool(name="raw", bufs=3))
        sD = ctx.enter_context(tc.tile_pool(name="sD", bufs=2))
        sP = ctx.enter_context(tc.tile_pool(name="sP", bufs=2))
        pD = ctx.enter_context(tc.tile_pool(name="pD", bufs=1, space="PSUM"))
        pP = ctx.enter_context(tc.tile_pool(name="pP", bufs=1, space="PSUM"))
        mkpool = ctx.enter_context(tc.tile_pool(name="mk", bufs=1, space="PSUM"))
        pnum = ctx.enter_context(tc.tile_pool(name="pnum", bufs=1, space="PSUM"))

        head_sb = singles.tile([T, HEAD_W], F32)
        nc.sync.dma_start(out=head_sb, in_=head_d.ap())
        trans_sb = head_sb[:, HEAD_TRANS:HEAD_TRANS + 128]
        biascol = head_sb[:, HEAD_BIAS:HEAD_BIAS + 2]
        bcolraw = head_sb[:, HEAD_BCOL:HEAD_BCOL + 3]
        initraw = head_sb[:, HEAD_INIT:HEAD_INIT + 128]

        e_bf = singles.tile([T, T], BF16)
        nc.scalar.activation(e_bf, trans_sb, AF.Exp)
        bcol_bf = singles.tile([T, 3], BF16)
        nc.scalar.activation(bcol_bf, bcolraw, AF.Exp)

        # ---- inits ---------------------------------------------------------
        stD = sD.tile([T, 384], BF16, name="aD")
        stP = sP.tile([T, 256], BF16, name="aP")
        nc.scalar.activation(stD[:, 0:128], initraw, AF.Exp, bias=biascol[:, 0:1])
        nc.sync.dma_start(out=stD[:, 128:384], in_=ones_d.ap())
        nc.sync.dma_start(out=stP, in_=ones_d.ap())

        # ---- xhat staging --------------------------------------------------
        xhat = singles.tile([T, NCOL], BF16)
        plan = []
        for off, sz in chunk_list(BASE_P, 768, 2304):
            plan.append(("D", off, sz, off / 384.0))
        for off, sz in chunk_list(NCOL - BASE_P, 512, 2048):
            plan.append(("P", BASE_P + off, sz, off / 256.0))
        plan.sort(key=lambda t: t[3])

        for _, off, sz, rr in plan:
            need = max(0.0, 3000.0 + rr * 1000.0 - 3500.0)
            with tc.tile_wait_until(need * 1e-6):
                rawc = raw.tile([T, sz], BF16, name="rawc")
                nc.sync.dma_start(out=rawc, in_=xt_d.ap()[:, off:off + sz])
                nc.scalar.activation(
                    xhat[:, off:off + sz], rawc, AF.Exp, bias=biascol[:, 1:2],
                )

        if numerator:
            # one-hot + emission tiles: [128, NB*8*128]; column block (b,k)
            # holds rows s = k*128..k*128+127 of batch row b (s%128 on the
            # partition axis)
            oh_sb = singles.tile([T, NB * 8 * T], FP8)
            oh2_sb = singles.tile([T, NB * 8 * T], FP8)
            xnb_sb = singles.tile([T, NB * 8 * T], FP8)
            ident_sb = singles.tile([T, T], F32)
            nc.sync.dma_start(out=ident_sb, in_=ident_d.ap())
            setags_sb = singles.tile([2 * NB, 1], I32)
            nc.sync.dma_start(out=setags_sb, in_=setags_d.ap())
            cp_ps = pnum.tile([T, T], F32)
            em_ps = pnum.tile([T, T], F32)
            se_sb = singles.tile([2 * NB, 1], F32)
            nwave = len(NUM_MS)
            per = NB * 8 * T // nwave
            for i in range(nwave):
                with tc.tile_wait_until(NUM_MS[i]):
                    sl = slice(i * per, (i + 1) * per)
                    nc.sync.dma_start(out=oh_sb[:, sl], in_=oh_d.ap()[:, sl])
                    nc.sync.dma_start(out=oh2_sb[:, sl], in_=oh2_d.ap()[:, sl])
                    nc.sync.dma_start(out=xnb_sb[:, sl], in_=xnb_d.ap()[:, sl])
            n_em = n_cp = 0
            for b in range(NB):
                with tc.tile_wait_until(NUMMM_MS[b]):
                    for k in range(8):
                        o = (b * 8 + k) * T
                        otile = oh_sb[:, o:o + T]
                        nc.tensor.matmul(
                            em_ps, lhsT=otile, rhs=xnb_sb[:, o:o + T],
                            start=(n_em == 0), stop=(n_em == NB * 8 - 1),
                            skip_group_check=True,
                        )
                        n_em += 1
                        nc.tensor.matmul(
                            cp_ps, lhsT=oh_sb[:, o:o + T],
                            rhs=oh2_sb[:, o:o + T],
                            start=(n_cp == 0), stop=(b == NB - 1 and k == 7),
                            skip_group_check=True,
                        )
                        n_cp += 1
            with tc.tile_wait_until(REDUCE_MS - 0.004):
                nc.gpsimd.indirect_dma_start(
                    out=se_sb, out_offset=None, in_=setable_d.ap(),
                    in_offset=bass.IndirectOffsetOnAxis(ap=setags_sb[:, 0:1], axis=0),
                )
                nc.sync.dma_start(out=outse_d.ap(), in_=se_sb)

        # ---- the two super-chains -----------------------------------------
        markers_sb = singles.tile([1, MKW], F32)
        nc.vector.memset(markers_sb, 0.0)
        mkps = mkpool.tile([1, MKW], F32)
        if chains:
            for r in range(max(R3, R2)):
                if r < R2:
                    ps = pP.tile([T, 256], F32, name="pP")
                    nc.tensor.matmul(ps, lhsT=e_bf, rhs=stP, start=True, stop=True)
                    nst = sP.tile([T, 256], BF16, name="aP")
                    nc.vector.tensor_tensor(
                        nst, ps, xhat[:, BASE_P + r * 256:BASE_P + (r + 1) * 256],
                        op=OP.mult,
                    )
                    stP = nst
                    if r == AR:
                        nc.tensor.matmul(
                            mkps[:, 512:768], lhsT=bcol_bf[:, 0:1],
                            rhs=stP, start=True, stop=True,
                        )
                    elif r == AR + 1:
                        nc.tensor.matmul(
                            mkps[:, 768:1024], lhsT=bcol_bf[:, 0:1],
                            rhs=stP, start=True, stop=True,
                        )
                    elif r == R2 - 1:
                        nc.tensor.matmul(
                            mkps[:, 1024:1152], lhsT=bcol_bf[:, 1:2],
                            rhs=stP[:, 0:128], start=True, stop=True,
                        )
                        nc.tensor.matmul(
                            mkps[:, 1152:1280], lhsT=bcol_bf[:, 2:3],
                            rhs=stP[:, 128:256], start=True, stop=True,
                        )
                if r >= R3:
                    continue
                psd = pD.tile([T, 384], F32, name="pD")
                nc.tensor.matmul(psd, lhsT=e_bf, rhs=stD, start=True, stop=True)
                nstd = sD.tile([T, 384], BF16, name="aD")
                nc.vector.tensor_tensor(
                    nstd, psd, xhat[:, r * 384:(r + 1) * 384], op=OP.mult,
                )
                stD = nstd
                if r == AR:
                    nc.tensor.matmul(
                        mkps[:, 0:384], lhsT=bcol_bf[:, 0:1],
                        rhs=stD, start=True, stop=True,
                    )
                elif r == R3 - 1:
                    nc.tensor.matmul(
                        mkps[:, 1536:1920], lhsT=bcol_bf[:, 0:1],
                        rhs=stD, start=True, stop=True,
                    )
            with tc.tile_wait_until(MARK1_MS):
                nc.scalar.activation(markers_sb[:, 0:384], mkps[:, 0:384], AF.Copy)
                nc.scalar.activation(markers_sb[:, 512:1024], mkps[:, 512:1024], AF.Copy)
            with tc.tile_wait_until(MARK2_MS):
                nc.scalar.activation(markers_sb[:, 1024:1280], mkps[:, 1024:1280], AF.Copy)
                nc.scalar.activation(markers_sb[:, 1536:1920], mkps[:, 1536:1920], AF.Copy)
                nc.sync.dma_start(out=markers_d.ap(), in_=markers_sb)

        if numerator:
            numred_sb = singles.tile([128, 2], F32)
            scr = singles.tile([128, T], F32)
            scr2 = singles.tile([128, T], F32)
            with tc.tile_wait_until(REDUCE_MS):
                nc.vector.tensor_tensor(scr, em_ps, ident_sb, op=OP.mult)
                nc.vector.reduce_sum(out=numred_sb[:, 0:1], in_=scr, axis=mybir.AxisListType.X)
                nc.vector.tensor_tensor(scr2, cp_ps, trans_sb, op=OP.mult)
                nc.vector.reduce_sum(out=numred_sb[:, 1:2], in_=scr2, axis=mybir.AxisListType.X)
                nc.sync.dma_start(out=numred_d.ap(), in_=numred_sb)

    nc.compile()
    return nc


def prepare_in_maps(inputs):
    x = np.asarray(inputs["inputs"], dtype=np.float32)
    tags = np.asarray(inputs["tags"]).astype(np.int64)
    trans = np.ascontiguousarray(np.asarray(inputs["transitions"], np.float32))
    start = np.asarray(inputs["start_transitions"], np.float32)
    end = np.asarray(inputs["end_transitions"], np.float32)
    x_bf = x.astype(NP_BF16)

    in_maps = []
    for c in range(8):
        head = np.zeros((T, HEAD_W), np.float32)
        head[:, HEAD_TRANS:HEAD_TRANS + 128] = trans
        head[:, HEAD_BIAS + 1] = -LEAK
        xt = np.zeros((T, NCOL), NP_BF16)
        for ch in CHAINS:
            if ch["core"] != c:
                continue
            s = ch["slot"]
            ps, R = ch["start"], ch["rounds"]
            blk = x_bf[:, ps:ps + R, :].transpose(2, 1, 0)   # [T, R, B]
            for r in range(R):
                o = col_off(s, r)
                xt[:, o:o + 128] = blk[:, r, :]
            if ch["kind"] == "first":
                head[:, HEAD_INIT:HEAD_INIT + 128] = x[:, 0, :].T
                head[:, HEAD_BIAS] = start
            if ch is CHAINS[-1]:
                head[:, HEAD_BCOL + 2] = end
        in_map = {"head": head, "xt": xt,
                  "ones_bf": np.ones((T, 256), NP_BF16)}

        tb = tags[c * NB:(c + 1) * NB]              # [16, 1024]
        oh = (tb[:, :, None] == np.arange(T)[None, None, :])
        tbn = np.concatenate([tb[:, 1:], np.full((NB, 1), -1, np.int64)], axis=1)
        oh2 = (tbn[:, :, None] == np.arange(T)[None, None, :])
        oh = oh.reshape(NB, 8, 128, T).transpose(2, 0, 1, 3).reshape(128, NB * 8 * T)
        oh2 = oh2.reshape(NB, 8, 128, T).transpose(2, 0, 1, 3).reshape(128, NB * 8 * T)
        xn = x[c * NB:(c + 1) * NB].reshape(NB, 8, 128, T)
        xn = xn.transpose(2, 0, 1, 3).reshape(128, NB * 8 * T)
        setags = np.concatenate(
            [tb[:, 0], T + tb[:, S - 1]]
        ).reshape(2 * NB, 1).astype(np.int32)
        in_map.update({
            "oh": np.ascontiguousarray(oh.astype(NP_FP8)),
            "oh2": np.ascontiguousarray(oh2.astype(NP_FP8)),
            "xnb": np.ascontiguousarray(xn.astype(NP_FP8)),
            "ident": np.eye(T, dtype=np.float32),
            "setable": np.concatenate([start, end]).reshape(2 * T, 1).astype(np.float32),
            "setags": setags,
        })
        in_maps.append(in_map)
    return in_maps


def assemble(results):
    den_tot = np.float64(0.0)
    for ch in CHAINS:
        mkf = results[ch["core"]]["markers"].reshape(MKW).astype(np.float64)
        s = ch["slot"]
        if s < 3:
            nA = mkf[128 * s:128 * (s + 1)]
            nB = mkf[1536 + 128 * s:1536 + 128 * (s + 1)]
        else:
            k = s - 3
            a0 = 512 + 128 * k
            a1 = 768 + 128 * k
            nA = mkf[a0:a0 + 128] if ch["a_round"] == AR else mkf[a1:a1 + 128]
            nB = mkf[1024 + 128 * k:1024 + 128 * (k + 1)]
        if ch["kind"] == "first":
            den_tot += (np.log(nB) + LEAK * ch["rounds"]).sum()
        else:
            nsteps = ch["rounds"] - 1 - ch["a_round"]
            den_tot += (np.log(nB) - np.log(nA) + LEAK * nsteps).sum()

    num_tot = np.float64(0.0)
    for c in range(8):
        num_tot += results[c]["numred"].astype(np.float64).sum()
        num_tot += results[c]["out_se"].astype(np.float64).sum()
    return np.asarray(num_tot - den_tot, dtype=np.float32)


_CACHE = {}


def kernel(**inputs):
    if "nc" not in _CACHE:
        _CACHE["nc"] = build_program()
    nc = _CACHE["nc"]
    in_maps = prepare_in_maps(inputs)
    res = bass_utils.run_bass_kernel_spmd(nc, in_maps, core_ids=list(range(8)))
    return assemble(res.results)


# revision 5
# speedup vs baseline: 1.2878x; 1.0086x over previous
"""Trainium2 Bass CRF loss — overlapping segments, fused super-chain variant.

Same algorithm as kernel_v2 (overlapping-segment forward recurrence in bf16
with a constant leak, boundary-sum markers, indirect-DMA numerator), but the
per-core chains are packed into TWO lockstep "super-chains" to cut
per-instruction overheads (PE wait-queue parking, DVE dispatch, Tile sem
throttling):

  super D: 3 segments side by side, state [128, 384], R3=41 rounds
  super P: 2 segments,              state [128, 256], R2=32 rounds

Each round is ONE matmul + ONE DVE multiply per super-chain (GPSIMD cannot
read PSUM — walrus rejects it — so all multiplies live on DVE; gpsimd only
runs the numerator gathers, fully overlapped under the chains).
"""

import os
import sys

for _p in ("/opt/trn_rl_repo", "/root/.axon_site/_ro/trn_rl_repo"):
    if os.path.isdir(_p) and _p not in sys.path:
        sys.path.append(_p)

from contextlib import ExitStack

import numpy as np

import concourse.bass as bass
import concourse.tile as tile
from concourse import bacc, mybir
from concourse import bass_utils

B, S, T = 128, 1024, 128
LEAK = 5.85
R3, R2 = 33, 34                 # rounds of the 3-seg / 2-seg super-chains
ROUNDS = [R3, R3, R3, R2, R2]   # per-segment-slot rounds (for the plan)
NCHAIN = 5
AR = 7
NCOL = R3 * 384 + R2 * 256
BASE_P = R3 * 384
NB = 16
TSE = T * T + 2 * T + 1
NUM_MS = [0.004 + 0.001 * i for i in range(16)]  # numerator DMA waves
NUMMM_MS = [0.014 + 0.0014 * i for i in range(16)]  # numerator matmul waves
REDUCE_MS = 0.038
MARK1_MS = 0.012
MARK2_MS = 0.036
MKW = 2048    # marker strip width: bank-aligned sections
# [0:384) A_D | [512:1024) A_P,A12_P | [1024:1280) B_P | [1536:1920) B_D

F32 = mybir.dt.float32
I32 = mybir.dt.int32
BF16 = mybir.dt.bfloat16
AF = mybir.ActivationFunctionType
OP = mybir.AluOpType
NP_BF16 = mybir.dt.np(BF16)
FP8 = mybir.dt.float8e4
NP_FP8 = mybir.dt.np(FP8)

HEAD_TRANS = 0
HEAD_BIAS = 128       # [init_bias, leak]
HEAD_BCOL = 130       # [ones, B_P0, B_P1] raw (pre-exp)
HEAD_INIT = 133
HEAD_W = 133 + 128


def chain_plan():
    chains = []
    for c in range(8):
        for s in range(NCHAIN):
            ch = {"core": c, "slot": s, "rounds": ROUNDS[s]}
            if c == 0 and s == 0:
                ch.update(kind="first", a_round=None, start=1, net=ROUNDS[0])
            else:
                ar = 8 if (s == NCHAIN - 1 and c == 7) else AR
                ch.update(kind="mid", a_round=ar, net=ROUNDS[s] - 1 - ar)
            chains.append(ch)
    pos = chains[0]["start"] + chains[0]["rounds"] - 1
    for ch in chains[1:]:
        ch["start"] = pos - ch["a_round"]
        pos = ch["start"] + ch["rounds"] - 1
    assert pos == S - 1, pos
    assert sum(ch["net"] for ch in chains) == S - 1
    return chains


CHAINS = chain_plan()


def col_off(slot, r):
    """xt column offset of the 128-col block for (segment slot, round)."""
    if slot < 3:
        return r * 384 + slot * 128
    return BASE_P + r * 256 + (slot - 3) * 128


def chunk_list(total, first, size):
    out = [(0, min(first, total))]
    while out[-1][0] + out[-1][1] < total:
        off = out[-1][0] + out[-1][1]
        out.append((off, min(size, total - off)))
    return out


def build_program(numerator=True, chains=True):
    nc = bacc.Bacc(
        "TRN2",
        target_bir_lowering=False,
        debug=False,
        enable_asserts=False,
        num_devices=8,
    )

    head_d = nc.dram_tensor("head", (T, HEAD_W), F32, kind="ExternalInput")
    xt_d = nc.dram_tensor("xt", (T, NCOL), BF16, kind="ExternalInput")
    ones_d = nc.dram_tensor("ones_bf", (T, 256), BF16, kind="ExternalInput")
    xnb_d = nc.dram_tensor("xnb", (T, NB * 8 * T), FP8, kind="ExternalInput")
    oh_d = nc.dram_tensor("oh", (T, NB * 8 * T), FP8, kind="ExternalInput")
    oh2_d = nc.dram_tensor("oh2", (T, NB * 8 * T), FP8, kind="ExternalInput")
    ident_d = nc.dram_tensor("ident", (T, T), F32, kind="ExternalInput")
    setable_d = nc.dram_tensor("setable", (2 * T, 1), F32, kind="ExternalInput")
    setags_d = nc.dram_tensor("setags", (2 * NB, 1), I32, kind="ExternalInput")

    markers_d = nc.dram_tensor("markers", (1, MKW), F32, kind="ExternalOutput")
    numred_d = nc.dram_tensor("numred", (128, 2), F32, kind="ExternalOutput")
    outse_d = nc.dram_tensor("out_se", (2 * NB, 1), F32, kind="ExternalOutput")

    with ExitStack() as ctx:
        tc = ctx.enter_context(tile.TileContext(nc))
        singles = ctx.enter_context(tc.tile_pool(name="singles", bufs=1))
        raw = ctx.enter_context(tc.tile_pool(name="raw", bufs=3))
        sD = ctx.enter_context(tc.tile_pool(name="sD", bufs=2))
        sP = ctx.enter_context(tc.tile_pool(name="sP", bufs=2))
        pD = ctx.enter_context(tc.tile_pool(name="pD", bufs=1, space="PSUM"))
        pP = ctx.enter_context(tc.tile_pool(name="pP", bufs=1, space="PSUM"))
        mkpool = ctx.enter_context(tc.tile_pool(name="mk", bufs=1, space="PSUM"))
        pnum = ctx.enter_context(tc.tile_pool(name="pnum", bufs=1, space="PSUM"))

        head_sb = singles.tile([T, HEAD_W], F32)
        nc.sync.dma_start(out=head_sb, in_=head_d.ap())
        trans_sb = head_sb[:, HEAD_TRANS:HEAD_TRANS + 128]
        biascol = head_sb[:, HEAD_BIAS:HEAD_BIAS + 2]
        bcolraw = head_sb[:, HEAD_BCOL:HEAD_BCOL + 3]
        initraw = head_sb[:, HEAD_INIT:HEAD_INIT + 128]

        e_bf = singles.tile([T, T], BF16)
        nc.scalar.activation(e_bf, trans_sb, AF.Exp)
        bcol_bf = singles.tile([T, 3], BF16)
        nc.scalar.activation(bcol_bf, bcolraw, AF.Exp)

        # ---- inits ---------------------------------------------------------
        stD = sD.tile([T, 384], BF16, name="aD")
        stP = sP.tile([T, 256], BF16, name="aP")
        nc.scalar.activation(stD[:, 0:128], initraw, AF.Exp, bias=biascol[:, 0:1])
        nc.sync.dma_start(out=stD[:, 128:384], in_=ones_d.ap())
        nc.sync.dma_start(out=stP, in_=ones_d.ap())

        # ---- xhat staging --------------------------------------------------
        xhat = singles.tile([T, NCOL], BF16)
        plan = []
        for off, sz in chunk_list(BASE_P, 1536, 2304):
            plan.append(("D", off, sz, off / 384.0))
        for off, sz in chunk_list(NCOL - BASE_P, 1024, 2048):
            plan.append(("P", BASE_P + off, sz, off / 256.0))
        plan.sort(key=lambda t: t[3])

        for _, off, sz, rr in plan:
            need = max(0.0, 3000.0 + rr * 1000.0 - 3500.0)
            with tc.tile_wait_until(need * 1e-6):
                rawc = raw.tile([T, sz], BF16, name="rawc")
                nc.sync.dma_start(out=rawc, in_=xt_d.ap()[:, off:off + sz])
                nc.scalar.activation(
                    xhat[:, off:off + sz], rawc, AF.Exp, bias=biascol[:, 1:2],
                )

        if numerator:
            # one-hot + emission tiles: [128, NB*8*128]; column block (b,k)
            # holds rows s = k*128..k*128+127 of batch row b (s%128 on the
            # partition axis)
            oh_sb = singles.tile([T, NB * 8 * T], FP8)
            oh2_sb = singles.tile([T, NB * 8 * T], FP8)
            xnb_sb = singles.tile([T, NB * 8 * T], FP8)
            ident_sb = singles.tile([T, T], F32)
            nc.sync.dma_start(out=ident_sb, in_=ident_d.ap())
            setags_sb = singles.tile([2 * NB, 1], I32)
            nc.sync.dma_start(out=setags_sb, in_=setags_d.ap())
            cp_ps = pnum.tile([T, T], F32)
            em_ps = pnum.tile([T, T], F32)
            se_sb = singles.tile([2 * NB, 1], F32)
            nwave = len(NUM_MS)
            per = NB * 8 * T // nwave
            for i in range(nwave):
                with tc.tile_wait_until(NUM_MS[i]):
                    sl = slice(i * per, (i + 1) * per)
                    nc.sync.dma_start(out=oh_sb[:, sl], in_=oh_d.ap()[:, sl])
                    nc.sync.dma_start(out=oh2_sb[:, sl], in_=oh2_d.ap()[:, sl])
                    nc.sync.dma_start(out=xnb_sb[:, sl], in_=xnb_d.ap()[:, sl])
            n_em = n_cp = 0
            for b in range(NB):
                with tc.tile_wait_until(NUMMM_MS[b]):
                    for k in range(8):
                        o = (b * 8 + k) * T
                        otile = oh_sb[:, o:o + T]
                        nc.tensor.matmul(
                            em_ps, lhsT=otile, rhs=xnb_sb[:, o:o + T],
                            start=(n_em == 0), stop=(n_em == NB * 8 - 1),
                            skip_group_check=True,
                        )
                        n_em += 1
                        nc.tensor.matmul(
                            cp_ps, lhsT=oh_sb[:, o:o + T],
                            rhs=oh2_sb[:, o:o + T],
                            start=(n_cp == 0), stop=(b == NB - 1 and k == 7),
                            skip_group_check=True,
                        )
                        n_cp += 1
            with tc.tile_wait_until(REDUCE_MS - 0.004):
                nc.gpsimd.indirect_dma_start(
                    out=se_sb, out_offset=None, in_=setable_d.ap(),
                    in_offset=bass.IndirectOffsetOnAxis(ap=setags_sb[:, 0:1], axis=0),
                )
                nc.sync.dma_start(out=outse_d.ap(), in_=se_sb)

        # ---- the two super-chains -----------------------------------------
        markers_sb = singles.tile([1, MKW], F32)
        nc.vector.memset(markers_sb, 0.0)
        mkps = mkpool.tile([1, MKW], F32)
        if chains:
            for r in range(max(R3, R2)):
                if r < R2:
                    ps = pP.tile([T, 256], F32, name="pP")
                    nc.tensor.matmul(ps, lhsT=e_bf, rhs=stP, start=True, stop=True)
                    nst = sP.tile([T, 256], BF16, name="aP")
                    nc.vector.tensor_tensor(
                        nst, ps, xhat[:, BASE_P + r * 256:BASE_P + (r + 1) * 256],
                        op=OP.mult,
                    )
                    stP = nst
                    if r == AR:
                        nc.tensor.matmul(
                            mkps[:, 512:768], lhsT=bcol_bf[:, 0:1],
                            rhs=stP, start=True, stop=True,
                        )
                    elif r == AR + 1:
                        nc.tensor.matmul(
                            mkps[:, 768:1024], lhsT=bcol_bf[:, 0:1],
                            rhs=stP, start=True, stop=True,
                        )
                    elif r == R2 - 1:
                        nc.tensor.matmul(
                            mkps[:, 1024:1152], lhsT=bcol_bf[:, 1:2],
                            rhs=stP[:, 0:128], start=True, stop=True,
                        )
                        nc.tensor.matmul(
                            mkps[:, 1152:1280], lhsT=bcol_bf[:, 2:3],
                            rhs=stP[:, 128:256], start=True, stop=True,
                        )
                if r >= R3:
                    continue
                psd = pD.tile([T, 384], F32, name="pD")
                nc.tensor.matmul(psd, lhsT=e_bf, rhs=stD, start=True, stop=True)
                nstd = sD.tile([T, 384], BF16, name="aD")
                nc.vector.tensor_tensor(
                    nstd, psd, xhat[:, r * 384:(r + 1) * 384], op=OP.mult,
                )
                stD = nstd
                if r == AR:
                    nc.tensor.matmul(
                        mkps[:, 0:384], lhsT=bcol_bf[:, 0:1],
                        rhs=stD, start=True, stop=True,
                    )
                elif r == R3 - 1:
                    nc.tensor.matmul(
                        mkps[:, 1536:1920], lhsT=bcol_bf[:, 0:1],
                        rhs=stD, start=True, stop=True,
                    )
            with tc.tile_wait_until(MARK1_MS):
                nc.scalar.activation(markers_sb[:, 0:384], mkps[:, 0:384], AF.Copy)
                nc.scalar.activation(markers_sb[:, 512:1024], mkps[:, 512:1024], AF.Copy)
            with tc.tile_wait_until(MARK2_MS):
                nc.scalar.activation(markers_sb[:, 1024:1280], mkps[:, 1024:1280], AF.Copy)
                nc.scalar.activation(markers_sb[:, 1536:1920], mkps[:, 1536:1920], AF.Copy)
                nc.sync.dma_start(out=markers_d.ap(), in_=markers_sb)

        if numerator:
            numred_sb = singles.tile([128, 2], F32)
            scr = singles.tile([128, T], F32)
            scr2 = singles.tile([128, T], F32)
            with tc.tile_wait_until(REDUCE_MS):
                nc.vector.tensor_tensor(scr, em_ps, ident_sb, op=OP.mult)
                nc.vector.reduce_sum(out=numred_sb[:, 0:1], in_=scr, axis=mybir.AxisListType.X)
                nc.vector.tensor_tensor(scr2, cp_ps, trans_sb, op=OP.mult)
                nc.vector.reduce_sum(out=numred_sb[:, 1:2], in_=scr2, axis=mybir.AxisListType.X)
                nc.sync.dma_start(out=numred_d.ap(), in_=numred_sb)

    nc.compile()
    return nc


def prepare_in_maps(inputs):
    x = np.asarray(inputs["inputs"], dtype=np.float32)
    tags = np.asarray(inputs["tags"]).astype(np.int64)
    trans = np.ascontiguousarray(np.asarray(inputs["transitions"], np.float32))
    start = np.asarray(inputs["start_transitions"], np.float32)
    end = np.asarray(inputs["end_transitions"], np.float32)
    x_bf = x.astype(NP_BF16)

    in_maps = []
    for c in range(8):
        head = np.zeros((T, HEAD_W), np.float32)
        head[:, HEAD_TRANS:HEAD_TRANS + 128] = trans
        head[:, HEAD_BIAS + 1] = -LEAK
        xt = np.zeros((T, NCOL), NP_BF16)
        for ch in CHAINS:
            if ch["core"] != c:
                continue
            s = ch["slot"]
            ps, R = ch["start"], ch["rounds"]
            blk = x_bf[:, ps:ps + R, :].transpose(2, 1, 0)   # [T, R, B]
            for r in range(R):
                o = col_off(s, r)
                xt[:, o:o + 128] = blk[:, r, :]
            if ch["kind"] == "first":
                head[:, HEAD_INIT:HEAD_INIT + 128] = x[:, 0, :].T
                head[:, HEAD_BIAS] = start
            if ch is CHAINS[-1]:
                head[:, HEAD_BCOL + 2] = end
        in_map = {"head": head, "xt": xt,
                  "ones_bf": np.ones((T, 256), NP_BF16)}

        tb = tags[c * NB:(c + 1) * NB]              # [16, 1024]
        oh = (tb[:, :, None] == np.arange(T)[None, None, :])
        tbn = np.concatenate([tb[:, 1:], np.full((NB, 1), -1, np.int64)], axis=1)
        oh2 = (tbn[:, :, None] == np.arange(T)[None, None, :])
        oh = oh.reshape(NB, 8, 128, T).transpose(2, 0, 1, 3).reshape(128, NB * 8 * T)
        oh2 = oh2.reshape(NB, 8, 128, T).transpose(2, 0, 1, 3).reshape(128, NB * 8 * T)
        xn = x[c * NB:(c + 1) * NB].reshape(NB, 8, 128, T)
        xn = xn.transpose(2, 0, 1, 3).reshape(128, NB * 8 * T)
        setags = np.concatenate(
            [tb[:, 0], T + tb[:, S - 1]]
        ).reshape(2 * NB, 1).astype(np.int32)
        in_map.update({
            "oh": np.ascontiguousarray(oh.astype(NP_FP8)),
            "oh2": np.ascontiguousarray(oh2.astype(NP_FP8)),
            "xnb": np.ascontiguousarray(xn.astype(NP_FP8)),
            "ident": np.eye(T, dtype=np.float32),
            "setable": np.concatenate([start, end]).reshape(2 * T, 1).astype(np.float32),
            "setags": setags,
        })
        in_maps.append(in_map)
    return in_maps


def assemble(results):
    den_tot = np.float64(0.0)
    for ch in CHAINS:
        mkf = results[ch["core"]]["markers"].reshape(MKW).astype(np.float64)
        s = ch["slot"]
        if s < 3:
            nA = mkf[128 * s:128 * (s + 1)]
            nB = mkf[1536 + 128 * s:1536 + 128 * (s + 1)]
        else:
            k = s - 3
            a0 = 512 + 128 * k
            a1 = 768 + 128 * k
            nA = mkf[a0:a0 + 128] if ch["a_round"] == AR else mkf[a1:a1 + 128]
            nB = mkf[1024 + 128 * k:1024 + 128 * (k + 1)]
        if ch["kind"] == "first":
            den_tot += (np.log(nB) + LEAK * ch["rounds"]).sum()
        else:
            nsteps = ch["rounds"] - 1 - ch["a_round"]
            den_tot += (np.log(nB) - np.log(nA) + LEAK * nsteps).sum()

    num_tot = np.float64(0.0)
    for c in range(8):
        num_tot += results[c]["numred"].astype(np.float64).sum()
        num_tot += results[c]["out_se"].astype(np.float64).sum()
    return np.asarray(num_tot - den_tot, dtype=np.float32)


_CACHE = {}


def kernel(**inputs):
    if "nc" not in _CACHE:
        _CACHE["nc"] = build_program()
    nc = _CACHE["nc"]
    in_maps = prepare_in_maps(inputs)
    res = bass_utils.run_bass_kernel_spmd(nc, in_maps, core_ids=list(range(8)))
    return assemble(res.results)


# revision 6
# speedup vs baseline: 1.3558x; 1.0528x over previous
"""Trainium2 Bass CRF loss — overlapping segments, fused super-chain variant.

Same algorithm as kernel_v2 (overlapping-segment forward recurrence in bf16
with a constant leak, boundary-sum markers, indirect-DMA numerator), but the
per-core chains are packed into TWO lockstep "super-chains" to cut
per-instruction overheads (PE wait-queue parking, DVE dispatch, Tile sem
throttling):

  super D: 3 segments side by side, state [128, 384], R3=41 rounds
  super P: 2 segments,              state [128, 256], R2=32 rounds

Each round is ONE matmul + ONE DVE multiply per super-chain (GPSIMD cannot
read PSUM — walrus rejects it — so all multiplies live on DVE; gpsimd only
runs the numerator gathers, fully overlapped under the chains).
"""

import os
import sys

for _p in ("/opt/trn_rl_repo", "/root/.axon_site/_ro/trn_rl_repo"):
    if os.path.isdir(_p) and _p not in sys.path:
        sys.path.append(_p)

from contextlib import ExitStack

import numpy as np

import concourse.bass as bass
import concourse.tile as tile
from concourse import bacc, mybir
from concourse import bass_utils

B, S, T = 128, 1024, 128
LEAK = 5.85
R3, R2 = 33, 34                 # rounds of the 3-seg / 2-seg super-chains
ROUNDS = [R3, R3, R3, R2, R2]   # per-segment-slot rounds (for the plan)
NCHAIN = 5
AR = 7
NCOL = R3 * 384 + R2 * 256
BASE_P = R3 * 384
NB = 16
TSE = T * T + 2 * T + 1
NUM_MS = [0.003 + 0.0007 * i for i in range(32)]  # numerator DMA waves
NUMMM_MS = [0.014 + 0.0014 * i for i in range(16)]  # numerator matmul waves
REDUCE_MS = 0.038
MARK1_MS = 0.012
MARK2_MS = 0.036
MKW = 2048    # marker strip width: bank-aligned sections
# [0:384) A_D | [512:1024) A_P,A12_P | [1024:1280) B_P | [1536:1920) B_D

F32 = mybir.dt.float32
I32 = mybir.dt.int32
BF16 = mybir.dt.bfloat16
AF = mybir.ActivationFunctionType
OP = mybir.AluOpType
NP_BF16 = mybir.dt.np(BF16)
FP8 = mybir.dt.float8e4
NP_FP8 = mybir.dt.np(FP8)

HEAD_TRANS = 0
HEAD_BIAS = 128       # [init_bias, leak]
HEAD_BCOL = 130       # [ones, B_P0, B_P1] raw (pre-exp)
HEAD_INIT = 133
HEAD_W = 133 + 128


def chain_plan():
    chains = []
    for c in range(8):
        for s in range(NCHAIN):
            ch = {"core": c, "slot": s, "rounds": ROUNDS[s]}
            if c == 0 and s == 0:
                ch.update(kind="first", a_round=None, start=1, net=ROUNDS[0])
            else:
                ar = 8 if (s == NCHAIN - 1 and c == 7) else AR
                ch.update(kind="mid", a_round=ar, net=ROUNDS[s] - 1 - ar)
            chains.append(ch)
    pos = chains[0]["start"] + chains[0]["rounds"] - 1
    for ch in chains[1:]:
        ch["start"] = pos - ch["a_round"]
        pos = ch["start"] + ch["rounds"] - 1
    assert pos == S - 1, pos
    assert sum(ch["net"] for ch in chains) == S - 1
    return chains


CHAINS = chain_plan()


def col_off(slot, r):
    """xt column offset of the 128-col block for (segment slot, round)."""
    if slot < 3:
        return r * 384 + slot * 128
    return BASE_P + r * 256 + (slot - 3) * 128


def chunk_list(total, first, size):
    out = [(0, min(first, total))]
    while out[-1][0] + out[-1][1] < total:
        off = out[-1][0] + out[-1][1]
        out.append((off, min(size, total - off)))
    return out


def build_program(numerator=True, chains=True):
    nc = bacc.Bacc(
        "TRN2",
        target_bir_lowering=False,
        debug=False,
        enable_asserts=False,
        num_devices=8,
    )

    head_d = nc.dram_tensor("head", (T, HEAD_W), F32, kind="ExternalInput")
    xt_d = nc.dram_tensor("xt", (T, NCOL), BF16, kind="ExternalInput")
    ones_d = nc.dram_tensor("ones_bf", (T, 256), BF16, kind="ExternalInput")
    nm_d = nc.dram_tensor("nm", (T, 3 * NB * 8 * T), FP8, kind="ExternalInput")
    ident_d = nc.dram_tensor("ident", (T, T), F32, kind="ExternalInput")
    setable_d = nc.dram_tensor("setable", (2 * T, 1), F32, kind="ExternalInput")
    setags_d = nc.dram_tensor("setags", (2 * NB, 1), I32, kind="ExternalInput")

    markers_d = nc.dram_tensor("markers", (1, MKW), F32, kind="ExternalOutput")
    numred_d = nc.dram_tensor("numred", (128, 2), F32, kind="ExternalOutput")
    outse_d = nc.dram_tensor("out_se", (2 * NB, 1), F32, kind="ExternalOutput")

    with ExitStack() as ctx:
        tc = ctx.enter_context(tile.TileContext(nc))
        singles = ctx.enter_context(tc.tile_pool(name="singles", bufs=1))
        raw = ctx.enter_context(tc.tile_pool(name="raw", bufs=3))
        sD = ctx.enter_context(tc.tile_pool(name="sD", bufs=2))
        sP = ctx.enter_context(tc.tile_pool(name="sP", bufs=2))
        pD = ctx.enter_context(tc.tile_pool(name="pD", bufs=1, space="PSUM"))
        pP = ctx.enter_context(tc.tile_pool(name="pP", bufs=1, space="PSUM"))
        mkpool = ctx.enter_context(tc.tile_pool(name="mk", bufs=1, space="PSUM"))
        pnum = ctx.enter_context(tc.tile_pool(name="pnum", bufs=1, space="PSUM"))

        head_sb = singles.tile([T, HEAD_W], F32)
        nc.sync.dma_start(out=head_sb, in_=head_d.ap())
        trans_sb = head_sb[:, HEAD_TRANS:HEAD_TRANS + 128]
        biascol = head_sb[:, HEAD_BIAS:HEAD_BIAS + 2]
        bcolraw = head_sb[:, HEAD_BCOL:HEAD_BCOL + 3]
        initraw = head_sb[:, HEAD_INIT:HEAD_INIT + 128]

        e_bf = singles.tile([T, T], BF16)
        nc.scalar.activation(e_bf, trans_sb, AF.Exp)
        bcol_bf = singles.tile([T, 3], BF16)
        nc.scalar.activation(bcol_bf, bcolraw, AF.Exp)

        # ---- inits ---------------------------------------------------------
        stD = sD.tile([T, 384], BF16, name="aD")
        stP = sP.tile([T, 256], BF16, name="aP")
        nc.scalar.activation(stD[:, 0:128], initraw, AF.Exp, bias=biascol[:, 0:1])
        nc.sync.dma_start(out=stD[:, 128:384], in_=ones_d.ap())
        nc.sync.dma_start(out=stP, in_=ones_d.ap())

        # ---- xhat staging --------------------------------------------------
        xhat = singles.tile([T, NCOL], BF16)
        plan = []
        for off, sz in chunk_list(BASE_P, 1536, 2304):
            plan.append(("D", off, sz, off / 384.0))
        for off, sz in chunk_list(NCOL - BASE_P, 1024, 2048):
            plan.append(("P", BASE_P + off, sz, off / 256.0))
        plan.sort(key=lambda t: t[3])

        for _, off, sz, rr in plan:
            need = max(0.0, 3000.0 + rr * 1000.0 - 3500.0)
            with tc.tile_wait_until(need * 1e-6):
                rawc = raw.tile([T, sz], BF16, name="rawc")
                nc.sync.dma_start(out=rawc, in_=xt_d.ap()[:, off:off + sz])
                nc.scalar.activation(
                    xhat[:, off:off + sz], rawc, AF.Exp, bias=biascol[:, 1:2],
                )

        if numerator:
            # one-hot + emission tiles: [128, NB*8*128]; column block (b,k)
            # holds rows s = k*128..k*128+127 of batch row b (s%128 on the
            # partition axis)
            nm_sb = singles.tile([T, 3 * NB * 8 * T], FP8)
            ident_sb = singles.tile([T, T], F32)
            nc.sync.dma_start(out=ident_sb, in_=ident_d.ap())
            setags_sb = singles.tile([2 * NB, 1], I32)
            nc.sync.dma_start(out=setags_sb, in_=setags_d.ap())
            cp_ps = pnum.tile([T, T], F32)
            em_ps = pnum.tile([T, T], F32)
            se_sb = singles.tile([2 * NB, 1], F32)
            nwave = len(NUM_MS)
            per = NB * 8 * T // nwave
            for i in range(nwave):
                with tc.tile_wait_until(NUM_MS[i]):
                    sl = slice(i * 3 * per, (i + 1) * 3 * per)
                    nc.sync.dma_start(out=nm_sb[:, sl], in_=nm_d.ap()[:, sl])

            def nmslice(o, which):
                # block at global col o lives in wave o // per at sub-offset
                w, r = divmod(o, per)
                return nm_sb[:, w * 3 * per + which * per + r:
                             w * 3 * per + which * per + r + T]
            n_em = n_cp = 0
            for b in range(NB):
                with tc.tile_wait_until(NUMMM_MS[b]):
                    for k in range(8):
                        o = (b * 8 + k) * T
                        otile = nmslice(o, 0)
                        nc.tensor.matmul(
                            em_ps, lhsT=otile, rhs=nmslice(o, 2),
                            start=(n_em == 0), stop=(n_em == NB * 8 - 1),
                            skip_group_check=True,
                        )
                        n_em += 1
                        nc.tensor.matmul(
                            cp_ps, lhsT=otile, rhs=nmslice(o, 1),
                            start=(n_cp == 0), stop=(b == NB - 1 and k == 7),
                            skip_group_check=True,
                        )
                        n_cp += 1
            with tc.tile_wait_until(REDUCE_MS - 0.004):
                nc.gpsimd.indirect_dma_start(
                    out=se_sb, out_offset=None, in_=setable_d.ap(),
                    in_offset=bass.IndirectOffsetOnAxis(ap=setags_sb[:, 0:1], axis=0),
                )
                nc.sync.dma_start(out=outse_d.ap(), in_=se_sb)

        # ---- the two super-chains -----------------------------------------
        markers_sb = singles.tile([1, MKW], F32)
        nc.vector.memset(markers_sb, 0.0)
        mkps = mkpool.tile([1, MKW], F32)
        if chains:
            for r in range(max(R3, R2)):
                if r < R2:
                    ps = pP.tile([T, 256], F32, name="pP")
                    nc.tensor.matmul(ps, lhsT=e_bf, rhs=stP, start=True, stop=True)
                    nst = sP.tile([T, 256], BF16, name="aP")
                    nc.vector.tensor_tensor(
                        nst, ps, xhat[:, BASE_P + r * 256:BASE_P + (r + 1) * 256],
                        op=OP.mult,
                    )
                    stP = nst
                    if r == AR:
                        nc.tensor.matmul(
                            mkps[:, 512:768], lhsT=bcol_bf[:, 0:1],
                            rhs=stP, start=True, stop=True,
                        )
                    elif r == AR + 1:
                        nc.tensor.matmul(
                            mkps[:, 768:1024], lhsT=bcol_bf[:, 0:1],
                            rhs=stP, start=True, stop=True,
                        )
                    elif r == R2 - 1:
                        nc.tensor.matmul(
                            mkps[:, 1024:1152], lhsT=bcol_bf[:, 1:2],
                            rhs=stP[:, 0:128], start=True, stop=True,
                        )
                        nc.tensor.matmul(
                            mkps[:, 1152:1280], lhsT=bcol_bf[:, 2:3],
                            rhs=stP[:, 128:256], start=True, stop=True,
                        )
                if r >= R3:
                    continue
                psd = pD.tile([T, 384], F32, name="pD")
                nc.tensor.matmul(psd, lhsT=e_bf, rhs=stD, start=True, stop=True)
                nstd = sD.tile([T, 384], BF16, name="aD")
                nc.vector.tensor_tensor(
                    nstd, psd, xhat[:, r * 384:(r + 1) * 384], op=OP.mult,
                )
                stD = nstd
                if r == AR:
                    nc.tensor.matmul(
                        mkps[:, 0:384], lhsT=bcol_bf[:, 0:1],
                        rhs=stD, start=True, stop=True,
                    )
                elif r == R3 - 1:
                    nc.tensor.matmul(
                        mkps[:, 1536:1920], lhsT=bcol_bf[:, 0:1],
                        rhs=stD, start=True, stop=True,
                    )
            with tc.tile_wait_until(MARK1_MS):
                nc.scalar.activation(markers_sb[:, 0:384], mkps[:, 0:384], AF.Copy)
                nc.scalar.activation(markers_sb[:, 512:1024], mkps[:, 512:1024], AF.Copy)
            with tc.tile_wait_until(MARK2_MS):
                nc.scalar.activation(markers_sb[:, 1024:1280], mkps[:, 1024:1280], AF.Copy)
                nc.scalar.activation(markers_sb[:, 1536:1920], mkps[:, 1536:1920], AF.Copy)
                nc.sync.dma_start(out=markers_d.ap(), in_=markers_sb)

        if numerator:
            numred_sb = singles.tile([128, 2], F32)
            scr = singles.tile([128, T], F32)
            scr2 = singles.tile([128, T], F32)
            with tc.tile_wait_until(REDUCE_MS):
                nc.vector.tensor_tensor(scr, em_ps, ident_sb, op=OP.mult)
                nc.vector.reduce_sum(out=numred_sb[:, 0:1], in_=scr, axis=mybir.AxisListType.X)
                nc.vector.tensor_tensor(scr2, cp_ps, trans_sb, op=OP.mult)
                nc.vector.reduce_sum(out=numred_sb[:, 1:2], in_=scr2, axis=mybir.AxisListType.X)
                nc.sync.dma_start(out=numred_d.ap(), in_=numred_sb)

    nc.compile()
    return nc


def prepare_in_maps(inputs):
    x = np.asarray(inputs["inputs"], dtype=np.float32)
    tags = np.asarray(inputs["tags"]).astype(np.int64)
    trans = np.ascontiguousarray(np.asarray(inputs["transitions"], np.float32))
    start = np.asarray(inputs["start_transitions"], np.float32)
    end = np.asarray(inputs["end_transitions"], np.float32)
    x_bf = x.astype(NP_BF16)

    in_maps = []
    for c in range(8):
        head = np.zeros((T, HEAD_W), np.float32)
        head[:, HEAD_TRANS:HEAD_TRANS + 128] = trans
        head[:, HEAD_BIAS + 1] = -LEAK
        xt = np.zeros((T, NCOL), NP_BF16)
        for ch in CHAINS:
            if ch["core"] != c:
                continue
            s = ch["slot"]
            ps, R = ch["start"], ch["rounds"]
            blk = x_bf[:, ps:ps + R, :].transpose(2, 1, 0)   # [T, R, B]
            for r in range(R):
                o = col_off(s, r)
                xt[:, o:o + 128] = blk[:, r, :]
            if ch["kind"] == "first":
                head[:, HEAD_INIT:HEAD_INIT + 128] = x[:, 0, :].T
                head[:, HEAD_BIAS] = start
            if ch is CHAINS[-1]:
                head[:, HEAD_BCOL + 2] = end
        in_map = {"head": head, "xt": xt,
                  "ones_bf": np.ones((T, 256), NP_BF16)}

        tb = tags[c * NB:(c + 1) * NB]              # [16, 1024]
        oh = (tb[:, :, None] == np.arange(T)[None, None, :])
        tbn = np.concatenate([tb[:, 1:], np.full((NB, 1), -1, np.int64)], axis=1)
        oh2 = (tbn[:, :, None] == np.arange(T)[None, None, :])
        oh = oh.reshape(NB, 8, 128, T).transpose(2, 0, 1, 3).reshape(128, NB * 8 * T)
        oh2 = oh2.reshape(NB, 8, 128, T).transpose(2, 0, 1, 3).reshape(128, NB * 8 * T)
        xn = x[c * NB:(c + 1) * NB].reshape(NB, 8, 128, T)
        xn = xn.transpose(2, 0, 1, 3).reshape(128, NB * 8 * T)
        setags = np.concatenate(
            [tb[:, 0], T + tb[:, S - 1]]
        ).reshape(2 * NB, 1).astype(np.int32)
        per = NB * 8 * T // len(NUM_MS)
        nm = np.zeros((T, 3 * NB * 8 * T), NP_FP8)
        for w in range(len(NUM_MS)):
            sl = slice(w * per, (w + 1) * per)
            nm[:, w * 3 * per:w * 3 * per + per] = oh.astype(NP_FP8)[:, sl]
            nm[:, w * 3 * per + per:w * 3 * per + 2 * per] = oh2.astype(NP_FP8)[:, sl]
            nm[:, w * 3 * per + 2 * per:w * 3 * per + 3 * per] = xn.astype(NP_FP8)[:, sl]
        in_map.update({
            "nm": nm,
            "ident": np.eye(T, dtype=np.float32),
            "setable": np.concatenate([start, end]).reshape(2 * T, 1).astype(np.float32),
            "setags": setags,
        })
        in_maps.append(in_map)
    return in_maps


def assemble(results):
    den_tot = np.float64(0.0)
    for ch in CHAINS:
        mkf = results[ch["core"]]["markers"].reshape(MKW).astype(np.float64)
        s = ch["slot"]
        if s < 3:
            nA = mkf[128 * s:128 * (s + 1)]
            nB = mkf[1536 + 128 * s:1536 + 128 * (s + 1)]
        else:
            k = s - 3
            a0 = 512 + 128 * k
            a1 = 768 + 128 * k
            nA = mkf[a0:a0 + 128] if ch["a_round"] == AR else mkf[a1:a1 + 128]
            nB = mkf[1024 + 128 * k:1024 + 128 * (k + 1)]
        if ch["kind"] == "first":
            den_tot += (np.log(nB) + LEAK * ch["rounds"]).sum()
        else:
            nsteps = ch["rounds"] - 1 - ch["a_round"]
            den_tot += (np.log(nB) - np.log(nA) + LEAK * nsteps).sum()

    num_tot = np.float64(0.0)
    for c in range(8):
        num_tot += results[c]["numred"].astype(np.float64).sum()
        num_tot += results[c]["out_se"].astype(np.float64).sum()
    return np.asarray(num_tot - den_tot, dtype=np.float32)


_CACHE = {}


def kernel(**inputs):
    if "nc" not in _CACHE:
        _CACHE["nc"] = build_program()
    nc = _CACHE["nc"]
    in_maps = prepare_in_maps(inputs)
    res = bass_utils.run_bass_kernel_spmd(nc, in_maps, core_ids=list(range(8)))
    return assemble(res.results)


# revision 7
# speedup vs baseline: 1.3639x; 1.0059x over previous
"""Trainium2 Bass CRF loss — overlapping segments, fused super-chain variant.

Same algorithm as kernel_v2 (overlapping-segment forward recurrence in bf16
with a constant leak, boundary-sum markers, indirect-DMA numerator), but the
per-core chains are packed into TWO lockstep "super-chains" to cut
per-instruction overheads (PE wait-queue parking, DVE dispatch, Tile sem
throttling):

  super D: 3 segments side by side, state [128, 384], R3=41 rounds
  super P: 2 segments,              state [128, 256], R2=32 rounds

Each round is ONE matmul + ONE DVE multiply per super-chain (GPSIMD cannot
read PSUM — walrus rejects it — so all multiplies live on DVE; gpsimd only
runs the numerator gathers, fully overlapped under the chains).
"""

import os
import sys

for _p in ("/opt/trn_rl_repo", "/root/.axon_site/_ro/trn_rl_repo"):
    if os.path.isdir(_p) and _p not in sys.path:
        sys.path.append(_p)

from contextlib import ExitStack

import numpy as np

import concourse.bass as bass
import concourse.tile as tile
from concourse import bacc, mybir
from concourse import bass_utils

B, S, T = 128, 1024, 128
LEAK = 5.85
R3, R2 = 33, 34                 # rounds of the 3-seg / 2-seg super-chains
ROUNDS = [R3, R3, R3, R2, R2]   # per-segment-slot rounds (for the plan)
NCHAIN = 5
AR = 7
NCOL = R3 * 384 + R2 * 256
BASE_P = R3 * 384
NB = 16
TSE = T * T + 2 * T + 1
NUM_MS = [0.008 + 0.0006 * i for i in range(32)]  # numerator DMA waves
NUMMM_MS = [0.014 + 0.0014 * i for i in range(16)]  # numerator matmul waves
REDUCE_MS = 0.038
MARK1_MS = 0.012
MARK2_MS = 0.036
MKW = 2048    # marker strip width: bank-aligned sections
# [0:384) A_D | [512:1024) A_P,A12_P | [1024:1280) B_P | [1536:1920) B_D

F32 = mybir.dt.float32
I32 = mybir.dt.int32
BF16 = mybir.dt.bfloat16
AF = mybir.ActivationFunctionType
OP = mybir.AluOpType
NP_BF16 = mybir.dt.np(BF16)
FP8 = mybir.dt.float8e4
NP_FP8 = mybir.dt.np(FP8)

HEAD_TRANS = 0
HEAD_BIAS = 128       # [init_bias, leak]
HEAD_BCOL = 130       # [ones, B_P0, B_P1] raw (pre-exp)
HEAD_INIT = 133
HEAD_W = 133 + 128


def chain_plan():
    chains = []
    for c in range(8):
        for s in range(NCHAIN):
            ch = {"core": c, "slot": s, "rounds": ROUNDS[s]}
            if c == 0 and s == 0:
                ch.update(kind="first", a_round=None, start=1, net=ROUNDS[0])
            else:
                ar = 8 if (s == NCHAIN - 1 and c == 7) else AR
                ch.update(kind="mid", a_round=ar, net=ROUNDS[s] - 1 - ar)
            chains.append(ch)
    pos = chains[0]["start"] + chains[0]["rounds"] - 1
    for ch in chains[1:]:
        ch["start"] = pos - ch["a_round"]
        pos = ch["start"] + ch["rounds"] - 1
    assert pos == S - 1, pos
    assert sum(ch["net"] for ch in chains) == S - 1
    return chains


CHAINS = chain_plan()


def col_off(slot, r):
    """xt column offset of the 128-col block for (segment slot, round)."""
    if slot < 3:
        return r * 384 + slot * 128
    return BASE_P + r * 256 + (slot - 3) * 128


def chunk_list(total, first, size):
    out = [(0, min(first, total))]
    while out[-1][0] + out[-1][1] < total:
        off = out[-1][0] + out[-1][1]
        out.append((off, min(size, total - off)))
    return out


def build_program(numerator=True, chains=True):
    nc = bacc.Bacc(
        "TRN2",
        target_bir_lowering=False,
        debug=False,
        enable_asserts=False,
        num_devices=8,
    )

    head_d = nc.dram_tensor("head", (T, HEAD_W), F32, kind="ExternalInput")
    xt_d = nc.dram_tensor("xt", (T, NCOL), BF16, kind="ExternalInput")
    ones_d = nc.dram_tensor("ones_bf", (T, 256), BF16, kind="ExternalInput")
    nm_d = nc.dram_tensor("nm", (T, 3 * NB * 8 * T), FP8, kind="ExternalInput")
    ident_d = nc.dram_tensor("ident", (T, T), F32, kind="ExternalInput")
    setable_d = nc.dram_tensor("setable", (2 * T, 1), F32, kind="ExternalInput")
    setags_d = nc.dram_tensor("setags", (2 * NB, 1), I32, kind="ExternalInput")

    markers_d = nc.dram_tensor("markers", (1, MKW), F32, kind="ExternalOutput")
    numred_d = nc.dram_tensor("numred", (128, 2), F32, kind="ExternalOutput")
    outse_d = nc.dram_tensor("out_se", (2 * NB, 1), F32, kind="ExternalOutput")

    with ExitStack() as ctx:
        tc = ctx.enter_context(tile.TileContext(nc))
        singles = ctx.enter_context(tc.tile_pool(name="singles", bufs=1))
        raw = ctx.enter_context(tc.tile_pool(name="raw", bufs=3))
        sD = ctx.enter_context(tc.tile_pool(name="sD", bufs=2))
        sP = ctx.enter_context(tc.tile_pool(name="sP", bufs=2))
        pD = ctx.enter_context(tc.tile_pool(name="pD", bufs=1, space="PSUM"))
        pP = ctx.enter_context(tc.tile_pool(name="pP", bufs=1, space="PSUM"))
        mkpool = ctx.enter_context(tc.tile_pool(name="mk", bufs=1, space="PSUM"))
        pnum = ctx.enter_context(tc.tile_pool(name="pnum", bufs=1, space="PSUM"))

        head_sb = singles.tile([T, HEAD_W], F32)
        nc.sync.dma_start(out=head_sb, in_=head_d.ap())
        trans_sb = head_sb[:, HEAD_TRANS:HEAD_TRANS + 128]
        biascol = head_sb[:, HEAD_BIAS:HEAD_BIAS + 2]
        bcolraw = head_sb[:, HEAD_BCOL:HEAD_BCOL + 3]
        initraw = head_sb[:, HEAD_INIT:HEAD_INIT + 128]

        e_bf = singles.tile([T, T], BF16)
        nc.scalar.activation(e_bf, trans_sb, AF.Exp)
        bcol_bf = singles.tile([T, 3], BF16)
        nc.scalar.activation(bcol_bf, bcolraw, AF.Exp)

        # ---- inits ---------------------------------------------------------
        stD = sD.tile([T, 384], BF16, name="aD")
        stP = sP.tile([T, 256], BF16, name="aP")
        nc.scalar.activation(stD[:, 0:128], initraw, AF.Exp, bias=biascol[:, 0:1])
        nc.sync.dma_start(out=stD[:, 128:384], in_=ones_d.ap())
        nc.sync.dma_start(out=stP, in_=ones_d.ap())

        # ---- xhat staging --------------------------------------------------
        xhat = singles.tile([T, NCOL], BF16)
        plan = []
        for off, sz in chunk_list(BASE_P, 1536, 2304):
            plan.append(("D", off, sz, off / 384.0))
        for off, sz in chunk_list(NCOL - BASE_P, 1024, 2048):
            plan.append(("P", BASE_P + off, sz, off / 256.0))
        plan.sort(key=lambda t: t[3])

        for _, off, sz, rr in plan:
            need = max(0.0, 3000.0 + rr * 1000.0 - 3500.0)
            with tc.tile_wait_until(need * 1e-6):
                rawc = raw.tile([T, sz], BF16, name="rawc")
                nc.sync.dma_start(out=rawc, in_=xt_d.ap()[:, off:off + sz])
                nc.scalar.activation(
                    xhat[:, off:off + sz], rawc, AF.Exp, bias=biascol[:, 1:2],
                )

        if numerator:
            # one-hot + emission tiles: [128, NB*8*128]; column block (b,k)
            # holds rows s = k*128..k*128+127 of batch row b (s%128 on the
            # partition axis)
            nm_sb = singles.tile([T, 3 * NB * 8 * T], FP8)
            ident_sb = singles.tile([T, T], F32)
            nc.sync.dma_start(out=ident_sb, in_=ident_d.ap())
            setags_sb = singles.tile([2 * NB, 1], I32)
            nc.sync.dma_start(out=setags_sb, in_=setags_d.ap())
            cp_ps = pnum.tile([T, T], F32)
            em_ps = pnum.tile([T, T], F32)
            se_sb = singles.tile([2 * NB, 1], F32)
            nwave = len(NUM_MS)
            per = NB * 8 * T // nwave
            for i in range(nwave):
                with tc.tile_wait_until(NUM_MS[i]):
                    sl = slice(i * 3 * per, (i + 1) * 3 * per)
                    nc.sync.dma_start(out=nm_sb[:, sl], in_=nm_d.ap()[:, sl])

            def nmslice(o, which):
                # block at global col o lives in wave o // per at sub-offset
                w, r = divmod(o, per)
                return nm_sb[:, w * 3 * per + which * per + r:
                             w * 3 * per + which * per + r + T]
            n_em = n_cp = 0
            for b in range(NB):
                with tc.tile_wait_until(NUMMM_MS[b]):
                    for k in range(8):
                        o = (b * 8 + k) * T
                        otile = nmslice(o, 0)
                        nc.tensor.matmul(
                            em_ps, lhsT=otile, rhs=nmslice(o, 2),
                            start=(n_em == 0), stop=(n_em == NB * 8 - 1),
                            skip_group_check=True,
                        )
                        n_em += 1
                        nc.tensor.matmul(
                            cp_ps, lhsT=otile, rhs=nmslice(o, 1),
                            start=(n_cp == 0), stop=(b == NB - 1 and k == 7),
                            skip_group_check=True,
                        )
                        n_cp += 1
            with tc.tile_wait_until(REDUCE_MS - 0.004):
                nc.gpsimd.indirect_dma_start(
                    out=se_sb, out_offset=None, in_=setable_d.ap(),
                    in_offset=bass.IndirectOffsetOnAxis(ap=setags_sb[:, 0:1], axis=0),
                )
                nc.sync.dma_start(out=outse_d.ap(), in_=se_sb)

        # ---- the two super-chains -----------------------------------------
        markers_sb = singles.tile([1, MKW], F32)
        nc.vector.memset(markers_sb, 0.0)
        mkps = mkpool.tile([1, MKW], F32)
        if chains:
            for r in range(max(R3, R2)):
                if r < R2:
                    ps = pP.tile([T, 256], F32, name="pP")
                    nc.tensor.matmul(ps, lhsT=e_bf, rhs=stP, start=True, stop=True)
                    nst = sP.tile([T, 256], BF16, name="aP")
                    nc.vector.tensor_tensor(
                        nst, ps, xhat[:, BASE_P + r * 256:BASE_P + (r + 1) * 256],
                        op=OP.mult,
                    )
                    stP = nst
                    if r == AR:
                        nc.tensor.matmul(
                            mkps[:, 512:768], lhsT=bcol_bf[:, 0:1],
                            rhs=stP, start=True, stop=True,
                        )
                    elif r == AR + 1:
                        nc.tensor.matmul(
                            mkps[:, 768:1024], lhsT=bcol_bf[:, 0:1],
                            rhs=stP, start=True, stop=True,
                        )
                    elif r == R2 - 1:
                        nc.tensor.matmul(
                            mkps[:, 1024:1152], lhsT=bcol_bf[:, 1:2],
                            rhs=stP[:, 0:128], start=True, stop=True,
                        )
                        nc.tensor.matmul(
                            mkps[:, 1152:1280], lhsT=bcol_bf[:, 2:3],
                            rhs=stP[:, 128:256], start=True, stop=True,
                        )
                if r >= R3:
                    continue
                psd = pD.tile([T, 384], F32, name="pD")
                nc.tensor.matmul(psd, lhsT=e_bf, rhs=stD, start=True, stop=True)
                nstd = sD.tile([T, 384], BF16, name="aD")
                nc.vector.tensor_tensor(
                    nstd, psd, xhat[:, r * 384:(r + 1) * 384], op=OP.mult,
                )
                stD = nstd
                if r == AR:
                    nc.tensor.matmul(
                        mkps[:, 0:384], lhsT=bcol_bf[:, 0:1],
                        rhs=stD, start=True, stop=True,
                    )
                elif r == R3 - 1:
                    nc.tensor.matmul(
                        mkps[:, 1536:1920], lhsT=bcol_bf[:, 0:1],
                        rhs=stD, start=True, stop=True,
                    )
            with tc.tile_wait_until(MARK1_MS):
                nc.scalar.activation(markers_sb[:, 0:384], mkps[:, 0:384], AF.Copy)
                nc.scalar.activation(markers_sb[:, 512:1024], mkps[:, 512:1024], AF.Copy)
            with tc.tile_wait_until(MARK2_MS):
                nc.scalar.activation(markers_sb[:, 1024:1280], mkps[:, 1024:1280], AF.Copy)
                nc.scalar.activation(markers_sb[:, 1536:1920], mkps[:, 1536:1920], AF.Copy)
                nc.sync.dma_start(out=markers_d.ap(), in_=markers_sb)

        if numerator:
            numred_sb = singles.tile([128, 2], F32)
            scr = singles.tile([128, T], F32)
            scr2 = singles.tile([128, T], F32)
            with tc.tile_wait_until(REDUCE_MS):
                nc.vector.tensor_tensor(scr, em_ps, ident_sb, op=OP.mult)
                nc.vector.reduce_sum(out=numred_sb[:, 0:1], in_=scr, axis=mybir.AxisListType.X)
                nc.vector.tensor_tensor(scr2, cp_ps, trans_sb, op=OP.mult)
                nc.vector.reduce_sum(out=numred_sb[:, 1:2], in_=scr2, axis=mybir.AxisListType.X)
                nc.sync.dma_start(out=numred_d.ap(), in_=numred_sb)

    nc.compile()
    return nc


def prepare_in_maps(inputs):
    x = np.asarray(inputs["inputs"], dtype=np.float32)
    tags = np.asarray(inputs["tags"]).astype(np.int64)
    trans = np.ascontiguousarray(np.asarray(inputs["transitions"], np.float32))
    start = np.asarray(inputs["start_transitions"], np.float32)
    end = np.asarray(inputs["end_transitions"], np.float32)
    x_bf = x.astype(NP_BF16)

    in_maps = []
    for c in range(8):
        head = np.zeros((T, HEAD_W), np.float32)
        head[:, HEAD_TRANS:HEAD_TRANS + 128] = trans
        head[:, HEAD_BIAS + 1] = -LEAK
        xt = np.zeros((T, NCOL), NP_BF16)
        for ch in CHAINS:
            if ch["core"] != c:
                continue
            s = ch["slot"]
            ps, R = ch["start"], ch["rounds"]
            blk = x_bf[:, ps:ps + R, :].transpose(2, 1, 0)   # [T, R, B]
            for r in range(R):
                o = col_off(s, r)
                xt[:, o:o + 128] = blk[:, r, :]
            if ch["kind"] == "first":
                head[:, HEAD_INIT:HEAD_INIT + 128] = x[:, 0, :].T
                head[:, HEAD_BIAS] = start
            if ch is CHAINS[-1]:
                head[:, HEAD_BCOL + 2] = end
        in_map = {"head": head, "xt": xt,
                  "ones_bf": np.ones((T, 256), NP_BF16)}

        tb = tags[c * NB:(c + 1) * NB]              # [16, 1024]
        oh = (tb[:, :, None] == np.arange(T)[None, None, :])
        tbn = np.concatenate([tb[:, 1:], np.full((NB, 1), -1, np.int64)], axis=1)
        oh2 = (tbn[:, :, None] == np.arange(T)[None, None, :])
        oh = oh.reshape(NB, 8, 128, T).transpose(2, 0, 1, 3).reshape(128, NB * 8 * T)
        oh2 = oh2.reshape(NB, 8, 128, T).transpose(2, 0, 1, 3).reshape(128, NB * 8 * T)
        xn = x[c * NB:(c + 1) * NB].reshape(NB, 8, 128, T)
        xn = xn.transpose(2, 0, 1, 3).reshape(128, NB * 8 * T)
        setags = np.concatenate(
            [tb[:, 0], T + tb[:, S - 1]]
        ).reshape(2 * NB, 1).astype(np.int32)
        per = NB * 8 * T // len(NUM_MS)
        nm = np.zeros((T, 3 * NB * 8 * T), NP_FP8)
        for w in range(len(NUM_MS)):
            sl = slice(w * per, (w + 1) * per)
            nm[:, w * 3 * per:w * 3 * per + per] = oh.astype(NP_FP8)[:, sl]
            nm[:, w * 3 * per + per:w * 3 * per + 2 * per] = oh2.astype(NP_FP8)[:, sl]
            nm[:, w * 3 * per + 2 * per:w * 3 * per + 3 * per] = xn.astype(NP_FP8)[:, sl]
        in_map.update({
            "nm": nm,
            "ident": np.eye(T, dtype=np.float32),
            "setable": np.concatenate([start, end]).reshape(2 * T, 1).astype(np.float32),
            "setags": setags,
        })
        in_maps.append(in_map)
    return in_maps


def assemble(results):
    den_tot = np.float64(0.0)
    for ch in CHAINS:
        mkf = results[ch["core"]]["markers"].reshape(MKW).astype(np.float64)
        s = ch["slot"]
        if s < 3:
            nA = mkf[128 * s:128 * (s + 1)]
            nB = mkf[1536 + 128 * s:1536 + 128 * (s + 1)]
        else:
            k = s - 3
            a0 = 512 + 128 * k
            a1 = 768 + 128 * k
            nA = mkf[a0:a0 + 128] if ch["a_round"] == AR else mkf[a1:a1 + 128]
            nB = mkf[1024 + 128 * k:1024 + 128 * (k + 1)]
        if ch["kind"] == "first":
            den_tot += (np.log(nB) + LEAK * ch["rounds"]).sum()
        else:
            nsteps = ch["rounds"] - 1 - ch["a_round"]
            den_tot += (np.log(nB) - np.log(nA) + LEAK * nsteps).sum()

    num_tot = np.float64(0.0)
    for c in range(8):
        num_tot += results[c]["numred"].astype(np.float64).sum()
        num_tot += results[c]["out_se"].astype(np.float64).sum()
    return np.asarray(num_tot - den_tot, dtype=np.float32)


_CACHE = {}


def kernel(**inputs):
    if "nc" not in _CACHE:
        _CACHE["nc"] = build_program()
    nc = _CACHE["nc"]
    in_maps = prepare_in_maps(inputs)
    res = bass_utils.run_bass_kernel_spmd(nc, in_maps, core_ids=list(range(8)))
    return assemble(res.results)


# revision 8
# speedup vs baseline: 1.3871x; 1.0171x over previous
"""Trainium2 Bass CRF loss — overlapping segments, fused super-chain variant.

Same algorithm as kernel_v2 (overlapping-segment forward recurrence in bf16
with a constant leak, boundary-sum markers, indirect-DMA numerator), but the
per-core chains are packed into TWO lockstep "super-chains" to cut
per-instruction overheads (PE wait-queue parking, DVE dispatch, Tile sem
throttling):

  super D: 3 segments side by side, state [128, 384], R3=41 rounds
  super P: 2 segments,              state [128, 256], R2=32 rounds

Each round is ONE matmul + ONE DVE multiply per super-chain (GPSIMD cannot
read PSUM — walrus rejects it — so all multiplies live on DVE; gpsimd only
runs the numerator gathers, fully overlapped under the chains).
"""

import os
import sys

for _p in ("/opt/trn_rl_repo", "/root/.axon_site/_ro/trn_rl_repo"):
    if os.path.isdir(_p) and _p not in sys.path:
        sys.path.append(_p)

from contextlib import ExitStack

import numpy as np

import concourse.bass as bass
import concourse.tile as tile
from concourse import bacc, mybir
from concourse import bass_utils

B, S, T = 128, 1024, 128
LEAK = 5.85
R3, R2 = 33, 34                 # rounds of the 3-seg / 2-seg super-chains
ROUNDS = [R3, R3, R3, R2, R2]   # per-segment-slot rounds (for the plan)
NCHAIN = 5
AR = 7
NCOL = R3 * 384 + R2 * 256
BASE_P = R3 * 384
NB = 16
TSE = T * T + 2 * T + 1
NUM_MS = [0.008 + 0.0006 * i for i in range(32)]  # numerator DMA waves
NUMMM_MS = [0.014 + 0.0014 * i for i in range(16)]  # numerator matmul waves
REDUCE_MS = 0.038
MARK1_MS = 0.012
MARK2_MS = 0.036
MKW = 2048    # marker strip width: bank-aligned sections
# [0:384) A_D | [512:1024) A_P,A12_P | [1024:1280) B_P | [1536:1920) B_D

F32 = mybir.dt.float32
I32 = mybir.dt.int32
BF16 = mybir.dt.bfloat16
AF = mybir.ActivationFunctionType
OP = mybir.AluOpType
NP_BF16 = mybir.dt.np(BF16)
FP8 = mybir.dt.float8e4
NP_FP8 = mybir.dt.np(FP8)

HEAD_TRANS = 0
HEAD_BIAS = 128       # [init_bias, leak]
HEAD_BCOL = 130       # [ones, B_P0, B_P1] raw (pre-exp)
HEAD_INIT = 133
HEAD_W = 133 + 128


def chain_plan():
    chains = []
    for c in range(8):
        for s in range(NCHAIN):
            ch = {"core": c, "slot": s, "rounds": ROUNDS[s]}
            if c == 0 and s == 0:
                ch.update(kind="first", a_round=None, start=1, net=ROUNDS[0])
            else:
                ar = 8 if (s == NCHAIN - 1 and c == 7) else AR
                ch.update(kind="mid", a_round=ar, net=ROUNDS[s] - 1 - ar)
            chains.append(ch)
    pos = chains[0]["start"] + chains[0]["rounds"] - 1
    for ch in chains[1:]:
        ch["start"] = pos - ch["a_round"]
        pos = ch["start"] + ch["rounds"] - 1
    assert pos == S - 1, pos
    assert sum(ch["net"] for ch in chains) == S - 1
    return chains


CHAINS = chain_plan()


def col_off(slot, r):
    """xt column offset of the 128-col block for (segment slot, round)."""
    if slot < 3:
        return r * 384 + slot * 128
    return BASE_P + r * 256 + (slot - 3) * 128


def chunk_list(total, first, size):
    out = [(0, min(first, total))]
    while out[-1][0] + out[-1][1] < total:
        off = out[-1][0] + out[-1][1]
        out.append((off, min(size, total - off)))
    return out


def build_program(numerator=True, chains=True):
    nc = bacc.Bacc(
        "TRN2",
        target_bir_lowering=False,
        debug=False,
        enable_asserts=False,
        num_devices=8,
    )

    head_d = nc.dram_tensor("head", (T, HEAD_W), F32, kind="ExternalInput")
    xt_d = nc.dram_tensor("xt", (T, NCOL), BF16, kind="ExternalInput")
    nm_d = nc.dram_tensor("nm", (T, 3 * NB * 8 * T), FP8, kind="ExternalInput")
    ident_d = nc.dram_tensor("ident", (T, T), F32, kind="ExternalInput")
    setable_d = nc.dram_tensor("setable", (2 * T, 1), F32, kind="ExternalInput")
    setags_d = nc.dram_tensor("setags", (2 * NB, 1), I32, kind="ExternalInput")

    markers_d = nc.dram_tensor("markers", (1, MKW), F32, kind="ExternalOutput")
    numred_d = nc.dram_tensor("numred", (128, 2), F32, kind="ExternalOutput")
    outse_d = nc.dram_tensor("out_se", (2 * NB, 1), F32, kind="ExternalOutput")

    with ExitStack() as ctx:
        tc = ctx.enter_context(tile.TileContext(nc))
        singles = ctx.enter_context(tc.tile_pool(name="singles", bufs=1))
        raw = ctx.enter_context(tc.tile_pool(name="raw", bufs=3))
        sD = ctx.enter_context(tc.tile_pool(name="sD", bufs=2))
        sP = ctx.enter_context(tc.tile_pool(name="sP", bufs=2))
        pD = ctx.enter_context(tc.tile_pool(name="pD", bufs=1, space="PSUM"))
        pP = ctx.enter_context(tc.tile_pool(name="pP", bufs=1, space="PSUM"))
        mkpool = ctx.enter_context(tc.tile_pool(name="mk", bufs=1, space="PSUM"))
        pnum = ctx.enter_context(tc.tile_pool(name="pnum", bufs=1, space="PSUM"))

        head_sb = singles.tile([T, HEAD_W], F32)
        nc.sync.dma_start(out=head_sb, in_=head_d.ap())
        trans_sb = head_sb[:, HEAD_TRANS:HEAD_TRANS + 128]
        biascol = head_sb[:, HEAD_BIAS:HEAD_BIAS + 2]
        bcolraw = head_sb[:, HEAD_BCOL:HEAD_BCOL + 3]
        initraw = head_sb[:, HEAD_INIT:HEAD_INIT + 128]

        e_bf = singles.tile([T, T], BF16)
        nc.scalar.activation(e_bf, trans_sb, AF.Exp)
        bcol_bf = singles.tile([T, 3], BF16)
        nc.scalar.activation(bcol_bf, bcolraw, AF.Exp)

        # ---- inits ---------------------------------------------------------
        stD = sD.tile([T, 384], BF16, name="aD")
        stP = sP.tile([T, 256], BF16, name="aP")
        nc.scalar.activation(stD[:, 0:128], initraw, AF.Exp, bias=biascol[:, 0:1])
        nc.scalar.activation(stD[:, 128:384], head_sb[:, 0:256], AF.Exp, scale=0.0)
        nc.scalar.activation(stP, head_sb[:, 0:256], AF.Exp, scale=0.0)

        # ---- xhat staging --------------------------------------------------
        xhat = singles.tile([T, NCOL], BF16)
        plan = []
        for off, sz in chunk_list(BASE_P, 1536, 2304):
            plan.append(("D", off, sz, off / 384.0))
        for off, sz in chunk_list(NCOL - BASE_P, 1024, 2048):
            plan.append(("P", BASE_P + off, sz, off / 256.0))
        plan.sort(key=lambda t: t[3])

        for _, off, sz, rr in plan:
            need = max(0.0, 3000.0 + rr * 1000.0 - 3500.0)
            with tc.tile_wait_until(need * 1e-6):
                rawc = raw.tile([T, sz], BF16, name="rawc")
                nc.sync.dma_start(out=rawc, in_=xt_d.ap()[:, off:off + sz])
                nc.scalar.activation(
                    xhat[:, off:off + sz], rawc, AF.Exp, bias=biascol[:, 1:2],
                )

        if numerator:
            # one-hot + emission tiles: [128, NB*8*128]; column block (b,k)
            # holds rows s = k*128..k*128+127 of batch row b (s%128 on the
            # partition axis)
            nm_sb = singles.tile([T, 3 * NB * 8 * T], FP8)
            ident_sb = singles.tile([T, T], F32)
            nc.sync.dma_start(out=ident_sb, in_=ident_d.ap())
            setags_sb = singles.tile([2 * NB, 1], I32)
            nc.sync.dma_start(out=setags_sb, in_=setags_d.ap())
            cp_ps = pnum.tile([T, T], F32)
            em_ps = pnum.tile([T, T], F32)
            se_sb = singles.tile([2 * NB, 1], F32)
            nwave = len(NUM_MS)
            per = NB * 8 * T // nwave
            for i in range(nwave):
                with tc.tile_wait_until(NUM_MS[i]):
                    sl = slice(i * 3 * per, (i + 1) * 3 * per)
                    nc.sync.dma_start(out=nm_sb[:, sl], in_=nm_d.ap()[:, sl])

            def nmslice(o, which):
                # block at global col o lives in wave o // per at sub-offset
                w, r = divmod(o, per)
                return nm_sb[:, w * 3 * per + which * per + r:
                             w * 3 * per + which * per + r + T]
            n_em = n_cp = 0
            for b in range(NB):
                with tc.tile_wait_until(NUMMM_MS[b]):
                    for k in range(8):
                        o = (b * 8 + k) * T
                        otile = nmslice(o, 0)
                        nc.tensor.matmul(
                            em_ps, lhsT=otile, rhs=nmslice(o, 2),
                            start=(n_em == 0), stop=(n_em == NB * 8 - 1),
                            skip_group_check=True,
                        )
                        n_em += 1
                        nc.tensor.matmul(
                            cp_ps, lhsT=otile, rhs=nmslice(o, 1),
                            start=(n_cp == 0), stop=(b == NB - 1 and k == 7),
                            skip_group_check=True,
                        )
                        n_cp += 1
            with tc.tile_wait_until(REDUCE_MS - 0.004):
                nc.gpsimd.indirect_dma_start(
                    out=se_sb, out_offset=None, in_=setable_d.ap(),
                    in_offset=bass.IndirectOffsetOnAxis(ap=setags_sb[:, 0:1], axis=0),
                )
                nc.sync.dma_start(out=outse_d.ap(), in_=se_sb)

        # ---- the two super-chains -----------------------------------------
        markers_sb = singles.tile([1, MKW], F32)
        nc.vector.memset(markers_sb, 0.0)
        mkps = mkpool.tile([1, MKW], F32)
        if chains:
            for r in range(max(R3, R2)):
                if r < R2:
                    ps = pP.tile([T, 256], F32, name="pP")
                    nc.tensor.matmul(ps, lhsT=e_bf, rhs=stP, start=True, stop=True)
                    nst = sP.tile([T, 256], BF16, name="aP")
                    nc.vector.tensor_tensor(
                        nst, ps, xhat[:, BASE_P + r * 256:BASE_P + (r + 1) * 256],
                        op=OP.mult,
                    )
                    stP = nst
                    if r == AR:
                        nc.tensor.matmul(
                            mkps[:, 512:768], lhsT=bcol_bf[:, 0:1],
                            rhs=stP, start=True, stop=True,
                        )
                    elif r == AR + 1:
                        nc.tensor.matmul(
                            mkps[:, 768:1024], lhsT=bcol_bf[:, 0:1],
                            rhs=stP, start=True, stop=True,
                        )
                    elif r == R2 - 1:
                        nc.tensor.matmul(
                            mkps[:, 1024:1152], lhsT=bcol_bf[:, 1:2],
                            rhs=stP[:, 0:128], start=True, stop=True,
                        )
                        nc.tensor.matmul(
                            mkps[:, 1152:1280], lhsT=bcol_bf[:, 2:3],
                            rhs=stP[:, 128:256], start=True, stop=True,
                        )
                if r >= R3:
                    continue
                psd = pD.tile([T, 384], F32, name="pD")
                nc.tensor.matmul(psd, lhsT=e_bf, rhs=stD, start=True, stop=True)
                nstd = sD.tile([T, 384], BF16, name="aD")
                nc.vector.tensor_tensor(
                    nstd, psd, xhat[:, r * 384:(r + 1) * 384], op=OP.mult,
                )
                stD = nstd
                if r == AR:
                    nc.tensor.matmul(
                        mkps[:, 0:384], lhsT=bcol_bf[:, 0:1],
                        rhs=stD, start=True, stop=True,
                    )
                elif r == R3 - 1:
                    nc.tensor.matmul(
                        mkps[:, 1536:1920], lhsT=bcol_bf[:, 0:1],
                        rhs=stD, start=True, stop=True,
                    )
            with tc.tile_wait_until(MARK1_MS):
                nc.scalar.activation(markers_sb[:, 0:384], mkps[:, 0:384], AF.Copy)
                nc.scalar.activation(markers_sb[:, 512:1024], mkps[:, 512:1024], AF.Copy)
            with tc.tile_wait_until(MARK2_MS):
                nc.scalar.activation(markers_sb[:, 1024:1280], mkps[:, 1024:1280], AF.Copy)
                nc.scalar.activation(markers_sb[:, 1536:1920], mkps[:, 1536:1920], AF.Copy)
                nc.sync.dma_start(out=markers_d.ap(), in_=markers_sb)

        if numerator:
            numred_sb = singles.tile([128, 2], F32)
            scr = singles.tile([128, T], F32)
            scr2 = singles.tile([128, T], F32)
            with tc.tile_wait_until(REDUCE_MS):
                nc.vector.tensor_tensor(scr, em_ps, ident_sb, op=OP.mult)
                nc.vector.reduce_sum(out=numred_sb[:, 0:1], in_=scr, axis=mybir.AxisListType.X)
                nc.vector.tensor_tensor(scr2, cp_ps, trans_sb, op=OP.mult)
                nc.vector.reduce_sum(out=numred_sb[:, 1:2], in_=scr2, axis=mybir.AxisListType.X)
                nc.sync.dma_start(out=numred_d.ap(), in_=numred_sb)

    nc.compile()
    return nc


def prepare_in_maps(inputs):
    x = np.asarray(inputs["inputs"], dtype=np.float32)
    tags = np.asarray(inputs["tags"]).astype(np.int64)
    trans = np.ascontiguousarray(np.asarray(inputs["transitions"], np.float32))
    start = np.asarray(inputs["start_transitions"], np.float32)
    end = np.asarray(inputs["end_transitions"], np.float32)
    x_bf = x.astype(NP_BF16)

    in_maps = []
    for c in range(8):
        head = np.zeros((T, HEAD_W), np.float32)
        head[:, HEAD_TRANS:HEAD_TRANS + 128] = trans
        head[:, HEAD_BIAS + 1] = -LEAK
        xt = np.zeros((T, NCOL), NP_BF16)
        for ch in CHAINS:
            if ch["core"] != c:
                continue
            s = ch["slot"]
            ps, R = ch["start"], ch["rounds"]
            blk = x_bf[:, ps:ps + R, :].transpose(2, 1, 0)   # [T, R, B]
            for r in range(R):
                o = col_off(s, r)
                xt[:, o:o + 128] = blk[:, r, :]
            if ch["kind"] == "first":
                head[:, HEAD_INIT:HEAD_INIT + 128] = x[:, 0, :].T
                head[:, HEAD_BIAS] = start
            if ch is CHAINS[-1]:
                head[:, HEAD_BCOL + 2] = end
        in_map = {"head": head, "xt": xt}

        tb = tags[c * NB:(c + 1) * NB]              # [16, 1024]
        oh = (tb[:, :, None] == np.arange(T)[None, None, :])
        tbn = np.concatenate([tb[:, 1:], np.full((NB, 1), -1, np.int64)], axis=1)
        oh2 = (tbn[:, :, None] == np.arange(T)[None, None, :])
        oh = oh.reshape(NB, 8, 128, T).transpose(2, 0, 1, 3).reshape(128, NB * 8 * T)
        oh2 = oh2.reshape(NB, 8, 128, T).transpose(2, 0, 1, 3).reshape(128, NB * 8 * T)
        xn = x[c * NB:(c + 1) * NB].reshape(NB, 8, 128, T)
        xn = xn.transpose(2, 0, 1, 3).reshape(128, NB * 8 * T)
        setags = np.concatenate(
            [tb[:, 0], T + tb[:, S - 1]]
        ).reshape(2 * NB, 1).astype(np.int32)
        per = NB * 8 * T // len(NUM_MS)
        nm = np.zeros((T, 3 * NB * 8 * T), NP_FP8)
        for w in range(len(NUM_MS)):
            sl = slice(w * per, (w + 1) * per)
            nm[:, w * 3 * per:w * 3 * per + per] = oh.astype(NP_FP8)[:, sl]
            nm[:, w * 3 * per + per:w * 3 * per + 2 * per] = oh2.astype(NP_FP8)[:, sl]
            nm[:, w * 3 * per + 2 * per:w * 3 * per + 3 * per] = xn.astype(NP_FP8)[:, sl]
        in_map.update({
            "nm": nm,
            "ident": np.eye(T, dtype=np.float32),
            "setable": np.concatenate([start, end]).reshape(2 * T, 1).astype(np.float32),
            "setags": setags,
        })
        in_maps.append(in_map)
    return in_maps


def assemble(results):
    den_tot = np.float64(0.0)
    for ch in CHAINS:
        mkf = results[ch["core"]]["markers"].reshape(MKW).astype(np.float64)
        s = ch["slot"]
        if s < 3:
            nA = mkf[128 * s:128 * (s + 1)]
            nB = mkf[1536 + 128 * s:1536 + 128 * (s + 1)]
        else:
            k = s - 3
            a0 = 512 + 128 * k
            a1 = 768 + 128 * k
            nA = mkf[a0:a0 + 128] if ch["a_round"] == AR else mkf[a1:a1 + 128]
            nB = mkf[1024 + 128 * k:1024 + 128 * (k + 1)]
        if ch["kind"] == "first":
            den_tot += (np.log(nB) + LEAK * ch["rounds"]).sum()
        else:
            nsteps = ch["rounds"] - 1 - ch["a_round"]
            den_tot += (np.log(nB) - np.log(nA) + LEAK * nsteps).sum()

    num_tot = np.float64(0.0)
    for c in range(8):
        num_tot += results[c]["numred"].astype(np.float64).sum()
        num_tot += results[c]["out_se"].astype(np.float64).sum()
    return np.asarray(num_tot - den_tot, dtype=np.float32)


_CACHE = {}


def kernel(**inputs):
    if "nc" not in _CACHE:
        _CACHE["nc"] = build_program()
    nc = _CACHE["nc"]
    in_maps = prepare_in_maps(inputs)
    res = bass_utils.run_bass_kernel_spmd(nc, in_maps, core_ids=list(range(8)))
    return assemble(res.results)


# revision 9
# speedup vs baseline: 1.3960x; 1.0064x over previous
"""Trainium2 Bass CRF loss — overlapping segments, fused super-chain variant.

Same algorithm as kernel_v2 (overlapping-segment forward recurrence in bf16
with a constant leak, boundary-sum markers, indirect-DMA numerator), but the
per-core chains are packed into TWO lockstep "super-chains" to cut
per-instruction overheads (PE wait-queue parking, DVE dispatch, Tile sem
throttling):

  super D: 3 segments side by side, state [128, 384], R3=41 rounds
  super P: 2 segments,              state [128, 256], R2=32 rounds

Each round is ONE matmul + ONE DVE multiply per super-chain (GPSIMD cannot
read PSUM — walrus rejects it — so all multiplies live on DVE; gpsimd only
runs the numerator gathers, fully overlapped under the chains).
"""

import os
import sys

for _p in ("/opt/trn_rl_repo", "/root/.axon_site/_ro/trn_rl_repo"):
    if os.path.isdir(_p) and _p not in sys.path:
        sys.path.append(_p)

from contextlib import ExitStack

import numpy as np

import concourse.bass as bass
import concourse.tile as tile
from concourse import bacc, mybir
from concourse import bass_utils

B, S, T = 128, 1024, 128
LEAK = 5.85
R3, R2 = 33, 34                 # rounds of the 3-seg / 2-seg super-chains
ROUNDS = [R3, R3, R3, R2, R2]   # per-segment-slot rounds (for the plan)
NCHAIN = 5
AR = 7
NCOL = R3 * 384 + R2 * 256
BASE_P = R3 * 384
NB = 16
TSE = T * T + 2 * T + 1
NUM_MS = [0.006 + 0.0006 * i for i in range(32)]  # numerator DMA waves
NUMMM_MS = [0.014 + 0.0014 * i for i in range(16)]  # numerator matmul waves
REDUCE_MS = 0.038
MARK1_MS = 0.012
MARK2_MS = 0.036
MKW = 2048    # marker strip width: bank-aligned sections
# [0:384) A_D | [512:1024) A_P,A12_P | [1024:1280) B_P | [1536:1920) B_D

F32 = mybir.dt.float32
I32 = mybir.dt.int32
BF16 = mybir.dt.bfloat16
AF = mybir.ActivationFunctionType
OP = mybir.AluOpType
NP_BF16 = mybir.dt.np(BF16)
FP8 = mybir.dt.float8e4
NP_FP8 = mybir.dt.np(FP8)

HEAD_TRANS = 0
HEAD_BIAS = 128       # [init_bias, leak]
HEAD_BCOL = 130       # [ones, B_P0, B_P1] raw (pre-exp)
HEAD_INIT = 133
HEAD_W = 133 + 128


def chain_plan():
    chains = []
    for c in range(8):
        for s in range(NCHAIN):
            ch = {"core": c, "slot": s, "rounds": ROUNDS[s]}
            if c == 0 and s == 0:
                ch.update(kind="first", a_round=None, start=1, net=ROUNDS[0])
            else:
                ar = 8 if (s == NCHAIN - 1 and c == 7) else AR
                ch.update(kind="mid", a_round=ar, net=ROUNDS[s] - 1 - ar)
            chains.append(ch)
    pos = chains[0]["start"] + chains[0]["rounds"] - 1
    for ch in chains[1:]:
        ch["start"] = pos - ch["a_round"]
        pos = ch["start"] + ch["rounds"] - 1
    assert pos == S - 1, pos
    assert sum(ch["net"] for ch in chains) == S - 1
    return chains


CHAINS = chain_plan()


def col_off(slot, r):
    """xt column offset of the 128-col block for (segment slot, round)."""
    if slot < 3:
        return r * 384 + slot * 128
    return BASE_P + r * 256 + (slot - 3) * 128


def chunk_list(total, first, size):
    out = [(0, min(first, total))]
    while out[-1][0] + out[-1][1] < total:
        off = out[-1][0] + out[-1][1]
        out.append((off, min(size, total - off)))
    return out


def build_program(numerator=True, chains=True):
    nc = bacc.Bacc(
        "TRN2",
        target_bir_lowering=False,
        debug=False,
        enable_asserts=False,
        num_devices=8,
    )

    head_d = nc.dram_tensor("head", (T, HEAD_W), F32, kind="ExternalInput")
    xt_d = nc.dram_tensor("xt", (T, NCOL), BF16, kind="ExternalInput")
    nm_d = nc.dram_tensor("nm", (T, 3 * NB * 8 * T), FP8, kind="ExternalInput")
    ident_d = nc.dram_tensor("ident", (T, T), F32, kind="ExternalInput")
    setable_d = nc.dram_tensor("setable", (2 * T, 1), F32, kind="ExternalInput")
    setags_d = nc.dram_tensor("setags", (2 * NB, 1), I32, kind="ExternalInput")

    markers_d = nc.dram_tensor("markers", (1, MKW), F32, kind="ExternalOutput")
    numred_d = nc.dram_tensor("numred", (128, 2), F32, kind="ExternalOutput")
    outse_d = nc.dram_tensor("out_se", (2 * NB, 1), F32, kind="ExternalOutput")

    with ExitStack() as ctx:
        tc = ctx.enter_context(tile.TileContext(nc))
        singles = ctx.enter_context(tc.tile_pool(name="singles", bufs=1))
        raw = ctx.enter_context(tc.tile_pool(name="raw", bufs=3))
        sD = ctx.enter_context(tc.tile_pool(name="sD", bufs=2))
        sP = ctx.enter_context(tc.tile_pool(name="sP", bufs=2))
        pD = ctx.enter_context(tc.tile_pool(name="pD", bufs=1, space="PSUM"))
        pP = ctx.enter_context(tc.tile_pool(name="pP", bufs=1, space="PSUM"))
        mkpool = ctx.enter_context(tc.tile_pool(name="mk", bufs=1, space="PSUM"))
        pnum = ctx.enter_context(tc.tile_pool(name="pnum", bufs=1, space="PSUM"))

        head_sb = singles.tile([T, HEAD_W], F32)
        nc.sync.dma_start(out=head_sb, in_=head_d.ap())
        trans_sb = head_sb[:, HEAD_TRANS:HEAD_TRANS + 128]
        biascol = head_sb[:, HEAD_BIAS:HEAD_BIAS + 2]
        bcolraw = head_sb[:, HEAD_BCOL:HEAD_BCOL + 3]
        initraw = head_sb[:, HEAD_INIT:HEAD_INIT + 128]

        e_bf = singles.tile([T, T], BF16)
        nc.scalar.activation(e_bf, trans_sb, AF.Exp)
        bcol_bf = singles.tile([T, 3], BF16)
        nc.scalar.activation(bcol_bf, bcolraw, AF.Exp)

        # ---- inits ---------------------------------------------------------
        stD = sD.tile([T, 384], BF16, name="aD")
        stP = sP.tile([T, 256], BF16, name="aP")
        nc.scalar.activation(stD[:, 0:128], initraw, AF.Exp, bias=biascol[:, 0:1])
        nc.scalar.activation(stD[:, 128:384], head_sb[:, 0:256], AF.Exp, scale=0.0)
        nc.scalar.activation(stP, head_sb[:, 0:256], AF.Exp, scale=0.0)

        # ---- xhat staging --------------------------------------------------
        xhat = singles.tile([T, NCOL], BF16)
        plan = []
        for off, sz in chunk_list(BASE_P, 1536, 2304):
            plan.append(("D", off, sz, off / 384.0))
        for off, sz in chunk_list(NCOL - BASE_P, 1024, 2048):
            plan.append(("P", BASE_P + off, sz, off / 256.0))
        plan.sort(key=lambda t: t[3])

        for _, off, sz, rr in plan:
            need = max(0.0, 2000.0 + rr * 1000.0 - 3500.0)
            with tc.tile_wait_until(need * 1e-6):
                rawc = raw.tile([T, sz], BF16, name="rawc")
                nc.sync.dma_start(out=rawc, in_=xt_d.ap()[:, off:off + sz])
                nc.scalar.activation(
                    xhat[:, off:off + sz], rawc, AF.Exp, bias=biascol[:, 1:2],
                )

        if numerator:
            # one-hot + emission tiles: [128, NB*8*128]; column block (b,k)
            # holds rows s = k*128..k*128+127 of batch row b (s%128 on the
            # partition axis)
            nm_sb = singles.tile([T, 3 * NB * 8 * T], FP8)
            ident_sb = singles.tile([T, T], F32)
            nc.sync.dma_start(out=ident_sb, in_=ident_d.ap())
            setags_sb = singles.tile([2 * NB, 1], I32)
            nc.sync.dma_start(out=setags_sb, in_=setags_d.ap())
            cp_ps = pnum.tile([T, T], F32)
            em_ps = pnum.tile([T, T], F32)
            se_sb = singles.tile([2 * NB, 1], F32)
            nwave = len(NUM_MS)
            per = NB * 8 * T // nwave
            for i in range(nwave):
                with tc.tile_wait_until(NUM_MS[i]):
                    sl = slice(i * 3 * per, (i + 1) * 3 * per)
                    nc.sync.dma_start(out=nm_sb[:, sl], in_=nm_d.ap()[:, sl])

            def nmslice(o, which):
                # block at global col o lives in wave o // per at sub-offset
                w, r = divmod(o, per)
                return nm_sb[:, w * 3 * per + which * per + r:
                             w * 3 * per + which * per + r + T]
            n_em = n_cp = 0
            for b in range(NB):
                with tc.tile_wait_until(NUMMM_MS[b]):
                    for k in range(8):
                        o = (b * 8 + k) * T
                        otile = nmslice(o, 0)
                        nc.tensor.matmul(
                            em_ps, lhsT=otile, rhs=nmslice(o, 2),
                            start=(n_em == 0), stop=(n_em == NB * 8 - 1),
                            skip_group_check=True,
                        )
                        n_em += 1
                        nc.tensor.matmul(
                            cp_ps, lhsT=otile, rhs=nmslice(o, 1),
                            start=(n_cp == 0), stop=(b == NB - 1 and k == 7),
                            skip_group_check=True,
                        )
                        n_cp += 1
            with tc.tile_wait_until(REDUCE_MS - 0.004):
                nc.gpsimd.indirect_dma_start(
                    out=se_sb, out_offset=None, in_=setable_d.ap(),
                    in_offset=bass.IndirectOffsetOnAxis(ap=setags_sb[:, 0:1], axis=0),
                )
                nc.sync.dma_start(out=outse_d.ap(), in_=se_sb)

        # ---- the two super-chains -----------------------------------------
        markers_sb = singles.tile([1, MKW], F32)
        nc.vector.memset(markers_sb, 0.0)
        mkps = mkpool.tile([1, MKW], F32)
        if chains:
            for r in range(max(R3, R2)):
                if r < R2:
                    ps = pP.tile([T, 256], F32, name="pP")
                    nc.tensor.matmul(ps, lhsT=e_bf, rhs=stP, start=True, stop=True)
                    nst = sP.tile([T, 256], BF16, name="aP")
                    nc.vector.tensor_tensor(
                        nst, ps, xhat[:, BASE_P + r * 256:BASE_P + (r + 1) * 256],
                        op=OP.mult,
                    )
                    stP = nst
                    if r == AR:
                        nc.tensor.matmul(
                            mkps[:, 512:768], lhsT=bcol_bf[:, 0:1],
                            rhs=stP, start=True, stop=True,
                        )
                    elif r == AR + 1:
                        nc.tensor.matmul(
                            mkps[:, 768:1024], lhsT=bcol_bf[:, 0:1],
                            rhs=stP, start=True, stop=True,
                        )
                    elif r == R2 - 1:
                        nc.tensor.matmul(
                            mkps[:, 1024:1152], lhsT=bcol_bf[:, 1:2],
                            rhs=stP[:, 0:128], start=True, stop=True,
                        )
                        nc.tensor.matmul(
                            mkps[:, 1152:1280], lhsT=bcol_bf[:, 2:3],
                            rhs=stP[:, 128:256], start=True, stop=True,
                        )
                if r >= R3:
                    continue
                psd = pD.tile([T, 384], F32, name="pD")
                nc.tensor.matmul(psd, lhsT=e_bf, rhs=stD, start=True, stop=True)
                nstd = sD.tile([T, 384], BF16, name="aD")
                nc.vector.tensor_tensor(
                    nstd, psd, xhat[:, r * 384:(r + 1) * 384], op=OP.mult,
                )
                stD = nstd
                if r == AR:
                    nc.tensor.matmul(
                        mkps[:, 0:384], lhsT=bcol_bf[:, 0:1],
                        rhs=stD, start=True, stop=True,
                    )
                elif r == R3 - 1:
                    nc.tensor.matmul(
                        mkps[:, 1536:1920], lhsT=bcol_bf[:, 0:1],
                        rhs=stD, start=True, stop=True,
                    )
            with tc.tile_wait_until(MARK1_MS):
                nc.scalar.activation(markers_sb[:, 0:384], mkps[:, 0:384], AF.Copy)
                nc.scalar.activation(markers_sb[:, 512:1024], mkps[:, 512:1024], AF.Copy)
            with tc.tile_wait_until(MARK2_MS):
                nc.scalar.activation(markers_sb[:, 1024:1280], mkps[:, 1024:1280], AF.Copy)
                nc.scalar.activation(markers_sb[:, 1536:1920], mkps[:, 1536:1920], AF.Copy)
                nc.sync.dma_start(out=markers_d.ap(), in_=markers_sb)

        if numerator:
            numred_sb = singles.tile([128, 2], F32)
            scr = singles.tile([128, T], F32)
            scr2 = singles.tile([128, T], F32)
            with tc.tile_wait_until(REDUCE_MS):
                nc.vector.tensor_tensor(scr, em_ps, ident_sb, op=OP.mult)
                nc.vector.reduce_sum(out=numred_sb[:, 0:1], in_=scr, axis=mybir.AxisListType.X)
                nc.vector.tensor_tensor(scr2, cp_ps, trans_sb, op=OP.mult)
                nc.vector.reduce_sum(out=numred_sb[:, 1:2], in_=scr2, axis=mybir.AxisListType.X)
                nc.sync.dma_start(out=numred_d.ap(), in_=numred_sb)

    nc.compile()
    return nc


def prepare_in_maps(inputs):
    x = np.asarray(inputs["inputs"], dtype=np.float32)
    tags = np.asarray(inputs["tags"]).astype(np.int64)
    trans = np.ascontiguousarray(np.asarray(inputs["transitions"], np.float32))
    start = np.asarray(inputs["start_transitions"], np.float32)
    end = np.asarray(inputs["end_transitions"], np.float32)
    x_bf = x.astype(NP_BF16)

    in_maps = []
    for c in range(8):
        head = np.zeros((T, HEAD_W), np.float32)
        head[:, HEAD_TRANS:HEAD_TRANS + 128] = trans
        head[:, HEAD_BIAS + 1] = -LEAK
        xt = np.zeros((T, NCOL), NP_BF16)
        for ch in CHAINS:
            if ch["core"] != c:
                continue
            s = ch["slot"]
            ps, R = ch["start"], ch["rounds"]
            blk = x_bf[:, ps:ps + R, :].transpose(2, 1, 0)   # [T, R, B]
            for r in range(R):
                o = col_off(s, r)
                xt[:, o:o + 128] = blk[:, r, :]
            if ch["kind"] == "first":
                head[:, HEAD_INIT:HEAD_INIT + 128] = x[:, 0, :].T
                head[:, HEAD_BIAS] = start
            if ch is CHAINS[-1]:
                head[:, HEAD_BCOL + 2] = end
        in_map = {"head": head, "xt": xt}

        tb = tags[c * NB:(c + 1) * NB]              # [16, 1024]
        oh = (tb[:, :, None] == np.arange(T)[None, None, :])
        tbn = np.concatenate([tb[:, 1:], np.full((NB, 1), -1, np.int64)], axis=1)
        oh2 = (tbn[:, :, None] == np.arange(T)[None, None, :])
        oh = oh.reshape(NB, 8, 128, T).transpose(2, 0, 1, 3).reshape(128, NB * 8 * T)
        oh2 = oh2.reshape(NB, 8, 128, T).transpose(2, 0, 1, 3).reshape(128, NB * 8 * T)
        xn = x[c * NB:(c + 1) * NB].reshape(NB, 8, 128, T)
        xn = xn.transpose(2, 0, 1, 3).reshape(128, NB * 8 * T)
        setags = np.concatenate(
            [tb[:, 0], T + tb[:, S - 1]]
        ).reshape(2 * NB, 1).astype(np.int32)
        per = NB * 8 * T // len(NUM_MS)
        nm = np.zeros((T, 3 * NB * 8 * T), NP_FP8)
        for w in range(len(NUM_MS)):
            sl = slice(w * per, (w + 1) * per)
            nm[:, w * 3 * per:w * 3 * per + per] = oh.astype(NP_FP8)[:, sl]
            nm[:, w * 3 * per + per:w * 3 * per + 2 * per] = oh2.astype(NP_FP8)[:, sl]
            nm[:, w * 3 * per + 2 * per:w * 3 * per + 3 * per] = xn.astype(NP_FP8)[:, sl]
        in_map.update({
            "nm": nm,
            "ident": np.eye(T, dtype=np.float32),
            "setable": np.concatenate([start, end]).reshape(2 * T, 1).astype(np.float32),
            "setags": setags,
        })
        in_maps.append(in_map)
    return in_maps


def assemble(results):
    den_tot = np.float64(0.0)
    for ch in CHAINS:
        mkf = results[ch["core"]]["markers"].reshape(MKW).astype(np.float64)
        s = ch["slot"]
        if s < 3:
            nA = mkf[128 * s:128 * (s + 1)]
            nB = mkf[1536 + 128 * s:1536 + 128 * (s + 1)]
        else:
            k = s - 3
            a0 = 512 + 128 * k
            a1 = 768 + 128 * k
            nA = mkf[a0:a0 + 128] if ch["a_round"] == AR else mkf[a1:a1 + 128]
            nB = mkf[1024 + 128 * k:1024 + 128 * (k + 1)]
        if ch["kind"] == "first":
            den_tot += (np.log(nB) + LEAK * ch["rounds"]).sum()
        else:
            nsteps = ch["rounds"] - 1 - ch["a_round"]
            den_tot += (np.log(nB) - np.log(nA) + LEAK * nsteps).sum()

    num_tot = np.float64(0.0)
    for c in range(8):
        num_tot += results[c]["numred"].astype(np.float64).sum()
        num_tot += results[c]["out_se"].astype(np.float64).sum()
    return np.asarray(num_tot - den_tot, dtype=np.float32)


_CACHE = {}


def kernel(**inputs):
    if "nc" not in _CACHE:
        _CACHE["nc"] = build_program()
    nc = _CACHE["nc"]
    in_maps = prepare_in_maps(inputs)
    res = bass_utils.run_bass_kernel_spmd(nc, in_maps, core_ids=list(range(8)))
    return assemble(res.results)
